# revision 34
# baseline (speedup 1.0000x reference)
"""DirPNAConv (gnn_message_passing) Trainium2 Bass kernel.

Math: for each direction, messages m_e = cat(x[recv], x[send]) @ preW + preb
split linearly into m_e = A[recv] + B[send] with per-node tables
A = x @ blockdiag(preW[:, :FI]) + preb, B = x @ blockdiag(preW[:, FI:]).
All four PNA aggregators (mean/min/max/std) then reduce to segment
reductions of B[send] over receivers:
  sum S, sumsq Q (A-terms cancel exactly in the variance),
  min/max shift by A[recv].

Sharding: per DIRECTION, nodes are sorted by that direction's degree
and dealt round-robin to the 8 cores. Every core computes the full
B tables locally from a replicated x — no collectives.

The per-edge B rows are fetched with dma_gather (int16 indices into a
pair-packed table btab[r] = [B[r] | B[r + 25088]], 256 B rows); a 3-op
f16 select with a HOST-PRE-EXPANDED mask picks the half. The gather's
Q7 descriptor generation (~7.5 ns/row) is the kernel's hard bottleneck,
so everything is organized to keep the Pool engine 100% busy:
  - per-tile (GS=1) ELL widths -> minimal padding (~2.5%),
  - tiles concatenated into ~16k-row supergroup gathers issued
    back-to-back (v tiles double-buffered; all indices preloaded),
  - the select mask is pre-expanded to [P, S, 64] on the host so the
    select runs at full DVE rate (no free-dim broadcast reads),
  - phase 0 (pair-table build) is deeply pipelined and everything else
    (select, trees, PE phase) hides under the gathers.

Per-tile pads repeat the tile's first slot so min/max are unaffected
and sums subtract padcount*first_slot.
"""

from contextlib import ExitStack

import numpy as np

import concourse.bacc as bacc
import concourse.bass_utils as bass_utils
import concourse.tile as tile
from concourse import bass, mybir
from concourse.masks import make_identity

F32 = mybir.dt.float32
F16 = mybir.dt.float16
I16 = mybir.dt.int16
ACTF = mybir.ActivationFunctionType
ALU = mybir.AluOpType
AXX = mybir.AxisListType.X

P = 128
D, T, FI = 64, 4, 16
AVG_LOG = float(np.log(17.0))
SG_CAP = 64                 # max slot-columns per supergroup gather


class CFG:
    n_nodes = 50000
    n_cores = 8

    @classmethod
    def derived(cls):
        cls.npc = (cls.n_nodes + cls.n_cores - 1) // cls.n_cores
        cls.npc_pad = ((cls.npc + P - 1) // P) * P
        cls.nt = cls.npc_pad // P
        cls.ntot = ((cls.n_nodes + 511) // 512) * 512
        cls.pair_rows = cls.ntot // 2
        cls.groups = [[t] for t in range(cls.nt)]


CFG.derived()


def configure(n_nodes, n_cores=8):
    CFG.n_nodes = n_nodes
    CFG.n_cores = n_cores
    CFG.derived()


def _supergroups(ks):
    """Consecutive equal-k blocks of <= SG_CAP columns (ks is already
    quantized to be constant within each block)."""
    sgs = []
    cur = []
    for t, k in enumerate(ks):
        if cur and (ks[cur[0]] != k or (len(cur) + 1) * k > SG_CAP):
            sgs.append(cur)
            cur = []
        cur.append(t)
    if cur:
        sgs.append(cur)
    return sgs


def _quantize(ks):
    """Pad per-tile widths (non-increasing) up to the first tile of each
    supergroup so every supergroup has one uniform k. Blocks are chosen
    by DP to minimize total padded columns (block cost = len * k_first,
    len <= SG_CAP // k_first)."""
    nt = len(ks)
    INF = 1 << 30
    dp = [INF] * (nt + 1)
    dp[nt] = 0
    choice = [1] * (nt + 1)
    for t in range(nt - 1, -1, -1):
        k0 = ks[t]
        maxlen = max(1, SG_CAP // k0)
        for L in range(1, min(maxlen, nt - t) + 1):
            c = L * k0 + dp[t + L]
            if c < dp[t]:
                dp[t] = c
                choice[t] = L
    out = []
    t = 0
    while t < nt:
        L = choice[t]
        out.extend([ks[t]] * L)
        t += L
    return out


# --------------------------------------------------------------------------
# Host-side routing prep (integer index manipulation only, no float math)
# --------------------------------------------------------------------------

def _core_edge_stats(recv, send, members, slot_of_global):
    npp = CFG.npc_pad
    sel = np.isin(recv, members)
    r = recv[sel]
    s = send[sel].astype(np.int64)
    slot = slot_of_global[r]
    order = np.argsort(slot, kind="stable")
    slot, s = slot[order], s[order]
    deg = np.bincount(slot, minlength=npp)
    start = np.zeros(npp, np.int64)
    start[1:] = np.cumsum(deg)[:-1]
    return slot, s, start, deg


def _wrap16(lst):
    assert lst.shape[0] % 16 == 0
    a = lst.astype(np.int16).reshape(-1, 16).T        # [16, S]
    return np.ascontiguousarray(np.tile(a, (8, 1)))   # [128, S]


def _host_prep(x, edge_index):
    src = np.asarray(edge_index[0]).astype(np.int64)
    dst = np.asarray(edge_index[1]).astype(np.int64)
    x = np.asarray(x, np.float32)
    nn, ncores, nt = CFG.n_nodes, CFG.n_cores, CFG.nt
    npp = CFG.npc_pad

    cnt_s2d_g = np.bincount(dst, minlength=nn)
    cnt_d2s_g = np.bincount(src, minlength=nn)
    orders = {"s": np.argsort(-cnt_s2d_g, kind="stable"),
              "d": np.argsort(-cnt_d2s_g, kind="stable")}
    rvsv = {"s": (dst, src), "d": (src, dst)}

    cores = []
    for c in range(ncores):
        co = {}
        for key in ("s", "d"):
            members = orders[key][c::ncores]
            glob_perm = np.full(npp, -1, np.int64)
            glob_perm[:members.shape[0]] = members
            slot_of_global = np.full(nn, -1, np.int64)
            slot_of_global[members] = np.arange(members.shape[0])
            co["glob_perm_" + key] = glob_perm
            rv, sv = rvsv[key]
            co["st_" + key] = _core_edge_stats(rv, sv, members,
                                               slot_of_global)
        cores.append(co)

    # per-tile uniform width = max degree over the tile, all cores;
    # then quantized so each supergroup gets one uniform k
    k_sched = {}
    for key in ("s", "d"):
        ks = []
        for t in range(nt):
            g0, g1 = t * P, (t + 1) * P
            kmax = 2
            for co in cores:
                _, _, _, deg = co["st_" + key]
                kmax = max(kmax, int(deg[g0:g1].max()))
            ks.append(kmax)
        k_sched[key] = _quantize(ks)

    for co in cores:
        for key in ("s", "d"):
            slot, s, start, deg = co.pop("st_" + key)
            kmax = max(k_sched[key])
            ell = np.full((npp, kmax), -1, np.int64)
            pos = np.arange(s.shape[0], dtype=np.int64) - start[slot]
            ell[slot, pos] = s
            first = ell[:, 0].copy()
            first[first < 0] = 0
            m = ell < 0
            ell[m] = np.broadcast_to(first[:, None], ell.shape)[m]
            idx_chunks, msk_chunks = [], []
            for t in range(nt):
                k = k_sched[key][t]
                blk = ell[t * P:(t + 1) * P, :k]          # [P, k]
                half = CFG.pair_rows
                idx_chunks.append(np.ascontiguousarray(
                    (blk % half).T).reshape(-1))          # [k, P] flat
                msk_chunks.append((blk >= half).T)        # [k, P]
            co["idx_" + key] = _wrap16(np.concatenate(idx_chunks))
            # expanded select mask: [P, Stot, 64] -> [P, Stot*64] f16
            mk = np.concatenate(msk_chunks, axis=0)       # [Stot, P]
            mke = np.repeat(mk.T.astype(np.float16)[:, :, None], 64,
                            axis=2)
            co["msk_" + key] = np.ascontiguousarray(
                mke.reshape(P, -1))                       # [128, Stot*64]
            degc = np.maximum(deg, 1).astype(np.float32)
            co["deg_" + key] = np.ascontiguousarray(
                degc.reshape(nt, P).T).astype(np.float32)     # [128, nt]
            sch = np.asarray(k_sched[key], np.int64)
            padc = (sch[:, None] - deg.reshape(nt, P)).T.astype(np.float32)
            co["pad_" + key] = np.ascontiguousarray(padc)     # [128, nt]
            xp = np.zeros((npp, D), np.float32)
            valid = co["glob_perm_" + key] >= 0
            xp[valid] = x[co["glob_perm_" + key][valid]]
            co["xperm_" + key] = xp

    xT = np.zeros((D, CFG.ntot), np.float16)
    xT[:, :nn] = x.T.astype(np.float16)
    return cores, k_sched, xT


def _blockdiag(w):  # w: [T, FI, FO] -> [T*FI, T*FO]
    t, fi, fo = w.shape
    out = np.zeros((t * fi, t * fo), np.float32)
    for i in range(t):
        out[i * fi:(i + 1) * fi, i * fo:(i + 1) * fo] = w[i]
    return out


def _weights_prep(inp):
    """Pure re-layout of the input weights (no arithmetic)."""
    w = {}
    for dk in ("s2d", "d2s"):
        preW = np.asarray(inp["pre_W_" + dk], np.float32)   # [T, 2FI, FI]
        preb = np.asarray(inp["pre_b_" + dk], np.float32).reshape(-1)  # [64]
        WA = _blockdiag(preW[:, :FI, :])                    # [64, 64]
        WB = _blockdiag(preW[:, FI:, :])                    # [64, 64]
        dup = np.zeros((65, 128), np.float32)
        dup[:64, :64] = WA
        dup[:64, 64:] = WA
        dup[64, :64] = preb
        dup[64, 64:] = preb
        half = np.zeros((65, 128), np.float32)
        half[:64, :64] = WA
        half[64, :64] = preb
        w["WAdup_" + dk] = dup
        w["WAhalf_" + dk] = half
        w["WB_" + dk] = WB
        postW = np.asarray(inp["post_W_" + dk], np.float32)  # [T, 208, 16]
        P0 = _blockdiag(postW[:, 0:FI, :])                   # [64, 64]
        Ps = []
        for blk in range(3):                                 # 1, amp, 1/amp
            Pg = np.zeros((256, 64), np.float32)
            for a in range(4):                               # mean/mn/mx/std
                for t in range(T):
                    rows = FI + blk * 4 * FI + a * FI
                    Pg[a * 64 + t * FI:a * 64 + (t + 1) * FI,
                       t * FI:(t + 1) * FI] = postW[t, rows:rows + FI, :]
            Ps.append(Pg)
        w["P0T_" + dk] = np.ascontiguousarray(P0.T)          # [64, 64]
        for i, Pg in enumerate(Ps):
            w[f"P{i+1}T_{dk}"] = np.ascontiguousarray(Pg.T)  # [64, 256]
        w["linW_" + dk] = np.asarray(inp["lin_W_" + dk], np.float32)
        w["linb_" + dk] = np.asarray(
            inp["lin_b_" + dk], np.float32).reshape(1, 64)
        w["postb_col_" + dk] = np.asarray(
            inp["post_b_" + dk], np.float32).reshape(64, 1)
    wbp = np.zeros((64, 128), np.float32)
    wbp[:, :64] = w["WB_s2d"]
    wbp[:, 64:] = w["WB_d2s"]
    w["WBpair"] = wbp
    w["selfW"] = np.asarray(inp["lin_self_W"], np.float32)
    w["selfb"] = np.asarray(inp["lin_self_b"], np.float32).reshape(1, 64)
    w["alpha"] = np.asarray(inp["alpha"], np.float32).reshape(1, 1)
    return w


# --------------------------------------------------------------------------
# Device kernel
# --------------------------------------------------------------------------

WEIGHT_SPECS = [
    ("WBpair", (64, 128)),
    ("WAdup_s2d", (65, 128)), ("WAdup_d2s", (65, 128)),
    ("WAhalf_s2d", (65, 128)), ("WAhalf_d2s", (65, 128)),
    ("P0T_s2d", (64, 64)), ("P0T_d2s", (64, 64)),
    ("P1T_s2d", (64, 256)), ("P1T_d2s", (64, 256)),
    ("P2T_s2d", (64, 256)), ("P2T_d2s", (64, 256)),
    ("P3T_s2d", (64, 256)), ("P3T_d2s", (64, 256)),
    ("linW_s2d", (64, 64)), ("linW_d2s", (64, 64)),
    ("linb_s2d", (1, 64)), ("linb_d2s", (1, 64)),
    ("postb_col_s2d", (64, 1)), ("postb_col_d2s", (64, 1)),
    ("selfW", (64, 64)), ("selfb", (1, 64)),
    ("alpha", (1, 1)),
]
COL_NAMES = ["deg", "pad"]


def _emit_tree(nc, pool, vsl, gw, k, out_f32, op, tag):
    """Run-wide min/max tree over vsl(a, b) -> AP [128, gw, b-a, 64]
    (f16). Overlap-pairing (idempotent ops) avoids odd-element carries."""
    if k == 1:
        nc.vector.tensor_copy(out=out_f32, in_=vsl(0, 1)[:, :, 0, :])
        return
    if k == 2:
        nc.vector.tensor_tensor(out=out_f32, in0=vsl(0, 1)[:, :, 0, :],
                                in1=vsl(1, 2)[:, :, 0, :], op=op)
        return
    h = (k + 1) // 2
    tmp = pool.tile([P, gw, max(2, (k + 1) // 2), 64], F16, tag=tag,
                    name=tag, bufs=1)
    nc.vector.tensor_tensor(out=tmp[:, :, :h, :], in0=vsl(0, h),
                            in1=vsl(k - h, k), op=op)
    m = h
    while m > 2:
        h = (m + 1) // 2
        nc.vector.tensor_tensor(out=tmp[:, :, :h, :], in0=tmp[:, :, :h, :],
                                in1=tmp[:, :, m - h:m, :], op=op)
        m = h
    nc.vector.tensor_tensor(out=out_f32, in0=tmp[:, :, 0, :],
                            in1=tmp[:, :, 1, :], op=op)


def _emit_sum_tree(nc, pool, first_in, k, out_f32, tag):
    """Run-wide exact sum tree over first_in(a, b) -> [P, gw, b-a, 64]
    (f16 source)."""
    gw = out_f32.shape[1]
    if k == 2:
        nc.vector.tensor_tensor(out=out_f32, in0=first_in(0, 1)[:, :, 0, :],
                                in1=first_in(1, 2)[:, :, 0, :], op=ALU.add)
        return
    if k == 3:
        nc.vector.tensor_tensor(out=out_f32, in0=first_in(0, 1)[:, :, 0, :],
                                in1=first_in(1, 2)[:, :, 0, :], op=ALU.add)
        nc.vector.tensor_tensor(out=out_f32, in0=out_f32,
                                in1=first_in(2, 3)[:, :, 0, :], op=ALU.add)
        return
    m = k // 2
    tmpb = pool.tile([P, gw, m, 64], F32, tag=tag, name=tag, bufs=1)
    nc.vector.tensor_tensor(out=tmpb[:, :, :m, :], in0=first_in(0, m),
                            in1=first_in(m, 2 * m), op=ALU.add)
    while m > 2:
        h, odd = m // 2, m % 2
        nc.vector.tensor_tensor(out=tmpb[:, :, :h, :], in0=tmpb[:, :, :h, :],
                                in1=tmpb[:, :, h + odd:m, :], op=ALU.add)
        m = h + odd
    nc.vector.tensor_tensor(out=out_f32, in0=tmpb[:, :, 0, :],
                            in1=tmpb[:, :, 1, :], op=ALU.add)
    if k % 2:
        nc.vector.tensor_tensor(out=out_f32, in0=out_f32,
                                in1=first_in(k - 1, k)[:, :, 0, :],
                                op=ALU.add)


def build_kernel(k_sched):
    nt, ntot, npc_pad = CFG.nt, CFG.ntot, CFG.npc_pad
    nc = bacc.Bacc("TRN2", target_bir_lowering=False, debug=False,
                   num_devices=CFG.n_cores)

    din = {}
    din["xT_f"] = nc.dram_tensor("xT_f", [64, ntot], F16,
                                 kind="ExternalInput").ap()
    y_dram = {}
    for key in ("s", "d"):
        din["xperm_" + key] = nc.dram_tensor(
            "xperm_" + key, [npc_pad, 64], F32, kind="ExternalInput").ap()
        stot = sum(k_sched[key])
        din["idx_" + key] = nc.dram_tensor(
            "idx_" + key, [P, 8 * stot], I16, kind="ExternalInput").ap()
        din["msk_" + key] = nc.dram_tensor(
            "msk_" + key, [P, stot * 64], F16, kind="ExternalInput").ap()
        for nm in COL_NAMES:
            din[f"{nm}_{key}"] = nc.dram_tensor(
                f"{nm}_{key}", [P, nt], F32, kind="ExternalInput").ap()
        y_dram[key] = nc.dram_tensor("y_" + key, [npc_pad, 64], F32,
                                     kind="ExternalOutput").ap()
    for nm, shp in WEIGHT_SPECS:
        din[nm] = nc.dram_tensor(nm, list(shp), F32,
                                 kind="ExternalInput").ap()
    btab = {
        "s": nc.dram_tensor("btab_s", [CFG.pair_rows, 128], F16,
                            kind="Internal").ap(),
        "d": nc.dram_tensor("btab_d", [CFG.pair_rows, 128], F16,
                            kind="Internal").ap(),
    }

    with tile.TileContext(nc) as tc:
        _emit(tc, nc, din, y_dram, btab, k_sched)

    nc.compile()
    return nc


def _emit(tc, nc, din, y_dram, btab, k_sched):
    nt, ntot = CFG.nt, CFG.ntot
    ctx = ExitStack()
    consts = ctx.enter_context(tc.tile_pool(name="consts", bufs=1))
    small = ctx.enter_context(tc.tile_pool(name="small", bufs=3))
    work = ctx.enter_context(tc.tile_pool(name="work", bufs=2))

    # ---- constants ------------------------------------------------------
    ident = consts.tile([P, P], F32)
    make_identity(nc, ident[:])
    eps_b = consts.tile([P, 1], F32)
    nc.vector.memset(eps_b[:], 1e-5)

    w_sb = {}
    for nm, shp in WEIGHT_SPECS:
        t = consts.tile([shp[0], shp[1]], F32, tag="w_" + nm)
        nc.sync.dma_start(out=t[:], in_=din[nm][:, :])
        w_sb[nm] = t

    # preload ALL gather indices (both directions) once
    idx_sb = {}
    for key in ("s", "d"):
        stot = sum(k_sched[key])
        it = consts.tile([P, 8 * stot], I16, tag="idx" + key,
                         name="idx" + key)
        nc.sync.dma_start(out=it[:], in_=din["idx_" + key][:, :])
        idx_sb[key] = it

    cols = {}
    amps, invamps, invdegs = {}, {}, {}
    for key in ("s", "d"):
        for nm in COL_NAMES:
            cname = f"{nm}_{key}"
            ct = consts.tile([P, nt], F32, tag=cname, name=cname)
            nc.sync.dma_start(out=ct[:], in_=din[cname][:, :])
            cols[cname] = ct
        amps[key] = consts.tile([P, nt], F32, tag="amp" + key,
                                name="amp" + key)
        nc.scalar.activation(out=amps[key][:], in_=cols["deg_" + key][:],
                             func=ACTF.Ln, bias=1.0, scale=1.0)
        invamps[key] = consts.tile([P, nt], F32, tag="iamp" + key,
                                   name="iamp" + key)
        nc.vector.reciprocal(out=invamps[key][:], in_=amps[key][:])
        invdegs[key] = consts.tile([P, nt], F32, tag="ideg" + key,
                                   name="ideg" + key)
        nc.vector.reciprocal(out=invdegs[key][:], in_=cols["deg_" + key][:])

    # ---- alpha, scaled linW, G matrices, bias ---------------------------
    alpha_b = consts.tile([64, 1], F32)
    nc.gpsimd.dma_start(
        out=alpha_b[:],
        in_=bass.AP(tensor=din["alpha"].tensor, offset=0,
                    ap=[[0, 64], [1, 1]]))
    a_d2s = alpha_b
    a_s2d = consts.tile([64, 1], F32)
    nc.vector.memset(a_s2d[:], 1.0)
    nc.vector.tensor_sub(out=a_s2d[:], in0=a_s2d[:], in1=alpha_b[:])

    alph = {"s": a_s2d, "d": a_d2s}
    dk_of = {"s": "s2d", "d": "d2s"}
    linWs = {}
    for key in ("s", "d"):
        lw = consts.tile([64, 64], F32, tag="linWs" + key, name="linWs" + key)
        nc.vector.tensor_scalar_mul(
            out=lw[:], in0=w_sb["linW_" + dk_of[key]][:], scalar1=alph[key][:])
        linWs[key] = lw

    G = {}
    G0 = {}
    selfW_ext = consts.tile([65, 64], F32)
    nc.sync.dma_start(out=selfW_ext[:64, :], in_=din["selfW"][:, :])

    wbpair16 = consts.tile([64, 128], F16, tag="wbpair16")
    nc.vector.tensor_copy(out=wbpair16[:], in_=w_sb["WBpair"][:])

    scale_of = {1: 1.0, 2: 1.0 / AVG_LOG, 3: AVG_LOG}
    with tc.tile_pool(name="setup_ps", bufs=4, space="PSUM") as setup_ps:
        # ---- phase 0: pair-packed B tables ------------------------------
        # btab_<dir>[r, :] = [B[r] | B[r + 25088]]; a 1024-node chunk
        # writes node-major with contiguous 128 B runs. Loads on sync,
        # matmuls on PE, f16 casts + table writes on ACT.
        CH = 1024
        n_chunks = ntot // CH
        half = CFG.pair_rows
        ph0_cm = tc.tile_pool(name="ph0", bufs=1)
        ph0 = ph0_cm.__enter__()
        # build btab_s for ALL nodes first (s-gathers can then start),
        # then btab_d underneath the early s-gathers.
        for key, p0 in (("s", 0), ("d", 64)):
            for ci in range(n_chunks):
                xch = ph0.tile([64, CH], F16, tag="xch", bufs=3)
                nc.scalar.dma_start(out=xch[:],
                                    in_=din["xT_f"][:, ci * CH:(ci + 1) * CH])
                ps_big = setup_ps.tile([P, CH // 2], F32, tag="bps",
                                       name="bps", bufs=2)
                for j in range(CH // P):
                    nc.tensor.matmul(out=ps_big[:, j * 64:(j + 1) * 64],
                                     lhsT=xch[:, j * P:(j + 1) * P],
                                     rhs=wbpair16[:, p0:p0 + 64],
                                     start=True, stop=True)
                nj = CH // P
                bsb = ph0.tile([P, nj, 64], F16, tag="bsb", bufs=3)
                nc.scalar.copy(
                    out=bsb[:, :, :],
                    in_=ps_big[:].rearrange("p (j c) -> p j c", j=nj))
                pieces = []
                n0 = ci * CH
                jmid = (half - n0) // P
                if jmid <= 0:
                    pieces.append((0, nj, n0 - half, 64))
                elif jmid >= nj:
                    pieces.append((0, nj, n0, 0))
                else:
                    pieces.append((0, jmid, n0, 0))
                    pieces.append((jmid, nj, n0 + jmid * P - half, 64))
                for j0, j1, row0, c0 in pieces:
                    out_ap = bass.AP(tensor=btab[key].tensor,
                                     offset=row0 * 128 + c0,
                                     ap=[[128, 128], [128 * 128, j1 - j0],
                                         [1, 64]])
                    nc.sync.dma_start(out=out_ap,
                                       in_=bsb[:, j0:j1, :])
        ph0_cm.__exit__(None, None, None)
        for key in ("s", "d"):
            dk = dk_of[key]
            for i in (1, 2, 3):
                for c in (0, 1):
                    ps = setup_ps.tile([P, 64], F32, tag="gps", name="gps",
                                       bufs=1)
                    nc.tensor.matmul(
                        out=ps[:],
                        lhsT=w_sb[f"P{i}T_{dk}"][:, c * P:(c + 1) * P],
                        rhs=linWs[key][:], start=True, stop=True)
                    g = consts.tile([P, 64], F32, tag=f"G{i}{key}{c}",
                                    name=f"G{i}{key}{c}")
                    nc.scalar.activation(out=g[:], in_=ps[:], func=ACTF.Copy,
                                         scale=scale_of[i])
                    G[f"{i}{key}{c}"] = g
            ps = setup_ps.tile([64, 64], F32, tag="g0ps", name="g0ps",
                             bufs=1)
            nc.tensor.matmul(out=ps[:], lhsT=w_sb[f"P0T_{dk}"][:],
                             rhs=linWs[key][:], start=True, stop=True)
            g0 = consts.tile([P, 64], F32, tag="G0" + key, name="G0" + key)
            nc.vector.tensor_copy(out=g0[:64, :], in_=ps[:])
            G0[key] = g0

        bias_ps = setup_ps.tile([1, 64], F32, tag="biasps",
                                name="biasps", bufs=1)
        nc.tensor.matmul(out=bias_ps[:], lhsT=w_sb["postb_col_s2d"][:],
                         rhs=linWs["s"][:], start=True, stop=False)
        nc.tensor.matmul(out=bias_ps[:], lhsT=w_sb["postb_col_d2s"][:],
                         rhs=linWs["d"][:], start=False, stop=True)
        tb = small.tile([1, 64], F32, tag="tb")
        nc.vector.tensor_scalar_mul(out=tb[:], in0=w_sb["linb_s2d"][:],
                                    scalar1=a_s2d[:1, :])
        nc.vector.tensor_add(out=tb[:], in0=tb[:], in1=bias_ps[:])
        tb2 = small.tile([1, 64], F32, tag="tb2")
        nc.vector.tensor_scalar_mul(out=tb2[:], in0=w_sb["linb_d2s"][:],
                                    scalar1=a_d2s[:1, :])
        nc.vector.tensor_add(out=tb[:], in0=tb[:], in1=tb2[:])
        nc.vector.tensor_add(out=tb[:], in0=tb[:], in1=w_sb["selfb"][:])
        nc.sync.dma_start(out=selfW_ext[64:65, :], in_=tb[:])

    # ---- main loop ------------------------------------------------------
    psum = ctx.enter_context(tc.tile_pool(name="psum", bufs=1, space="PSUM"))


    def bcast(col_ap, gw):
        # [128, gw] column slice -> [128, gw, 64] free-broadcast AP
        return col_ap.unsqueeze(2).to_broadcast([P, gw, 64])

    pend = []

    def _emit_pe(item):
        key, tiles, ag, k = item
        for ti, t in enumerate(tiles):
            xp = small.tile([P, 64], F32, tag="xp")
            nc.sync.dma_start(
                out=xp[:],
                in_=din["xperm_" + key][t * P:(t + 1) * P, :])
            xpT_ps = psum.tile([64, P], F32, tag="tp",
                               name="xpT_ps", bufs=2)
            nc.tensor.transpose(out=xpT_ps[:], in_=xp[:],
                                identity=ident[:])
            xpT32 = small.tile([65, P], F32, tag="xpT32")
            nc.scalar.copy(out=xpT32[:64, :], in_=xpT_ps[:])
            nc.vector.memset(xpT32[64:65, :], 1.0)

            u1 = psum.tile([64, P], F32, tag="u1", name="u1" + key,
                           bufs=2)
            aggT = work.tile([P, 2, P], F32, tag="aggT",
                             name="aggT" + key)
            for c, wkind in ((0, "dup"), (1, "half")):
                tp = psum.tile([P, P], F32, tag="tp", name="tp",
                               bufs=2)
                nc.tensor.matmul(out=tp[:],
                                 lhsT=ag[:, ti, 2 * c:2 * c + 2, :],
                                 rhs=ident[:], is_transpose=True,
                                 start=True, stop=False,
                                 skip_group_check=True)
                nc.tensor.matmul(
                    out=tp[:],
                    lhsT=w_sb[f"WA{wkind}_{dk_of[key]}"][:],
                    rhs=xpT32[:], start=False, stop=True,
                    skip_group_check=True)
                nc.scalar.copy(out=aggT[:, c, :], in_=tp[:])

            nc.tensor.matmul(out=u1[:], lhsT=G[f"1{key}0"][:],
                             rhs=aggT[:, 0, :], start=True,
                             stop=False, skip_group_check=True)
            nc.tensor.matmul(out=u1[:], lhsT=G[f"1{key}1"][:],
                             rhs=aggT[:, 1, :], start=False,
                             stop=False, skip_group_check=True)
            nc.tensor.matmul(out=u1[:], lhsT=G0[key][:64, :],
                             rhs=xpT32[:64, :], start=False,
                             stop=(key == "d"),
                             skip_group_check=True)
            if key == "s":
                nc.tensor.matmul(out=u1[:], lhsT=selfW_ext[:],
                                 rhs=xpT32[:], start=False,
                                 stop=True, skip_group_check=True)
            u23 = psum.tile([P, P], F32, tag="u23",
                            name="u23" + key, bufs=2)
            nc.tensor.matmul(out=u23[:64, :], lhsT=G[f"2{key}0"][:],
                             rhs=aggT[:, 0, :], start=True,
                             stop=False, skip_group_check=True)
            nc.tensor.matmul(out=u23[:64, :], lhsT=G[f"2{key}1"][:],
                             rhs=aggT[:, 1, :], start=False,
                             stop=True, skip_group_check=True)
            nc.tensor.matmul(out=u23[64:, :], lhsT=G[f"3{key}0"][:],
                             rhs=aggT[:, 0, :], start=True,
                             stop=False, skip_group_check=True)
            nc.tensor.matmul(out=u23[64:, :], lhsT=G[f"3{key}1"][:],
                             rhs=aggT[:, 1, :], start=False,
                             stop=True, skip_group_check=True)

            u1sb = small.tile([64, P], F32, tag="u1sb")
            nc.scalar.copy(out=u1sb[:], in_=u1[:])
            u1T = psum.tile([P, 64], F32, tag="utr", name="u1T",
                            bufs=2)
            nc.tensor.transpose(out=u1T[:], in_=u1sb[:],
                                identity=ident[:64, :64])
            upk = small.tile([P, P], F32, tag="upk",
                             name="upk" + key)
            nc.scalar.copy(out=upk[:], in_=u23[:])
            uT = psum.tile([P, P], F32, tag="utr", name="uT" + key,
                           bufs=2)
            nc.tensor.transpose(out=uT[:], in_=upk[:],
                                identity=ident[:])
            y_sb = small.tile([P, 64], F32, tag="y_sb")
            sc = small.tile([P, 64], F32, tag="sc", name="sc" + key)
            nc.scalar.activation(out=sc[:], in_=uT[:, 0:64],
                                 func=ACTF.Copy,
                                 scale=amps[key][:, t:t + 1])
            nc.vector.tensor_add(out=y_sb[:], in0=u1T[:], in1=sc[:])
            nc.scalar.activation(out=sc[:], in_=uT[:, 64:128],
                                 func=ACTF.Copy,
                                 scale=invamps[key][:, t:t + 1])
            nc.vector.tensor_add(out=y_sb[:], in0=y_sb[:],
                                 in1=sc[:])
            nc.sync.dma_start(
                out=y_dram[key][t * P:(t + 1) * P, :], in_=y_sb[:])

    for key in ("s", "d"):
        sgs = _supergroups(k_sched[key])
        offs = []
        o = 0
        for sg in sgs:
            offs.append(o)
            o += sum(k_sched[key][t] for t in sg)
        # descending-k pipeline, but finish with the few-tile head groups
        # so the post-gather tail (trees + PE phase) is short
        order = list(zip(sgs, offs))
        order = order[2:] + order[:2][::-1]
        for sg, off in order:
            S = sum(k_sched[key][t] for t in sg)
            v = work.tile([P, S, 128], F16, tag="vg", name="vg" + key,
                          bufs=3)
            nc.gpsimd.dma_gather(
                out_ap=v[:, :, :], in_ap=btab[key][:, :],
                idxs_ap=idx_sb[key][:, 8 * off:8 * (off + S)],
                num_idxs=P * S, num_idxs_reg=P * S, elem_size=128,
                single_packet=False)

            mskt = work.tile([P, S, 64], F16, tag="msk", name="msk" + key,
                             bufs=3)
            nc.sync.dma_start(
                out=mskt[:],
                in_=din["msk_" + key][:, off * 64:(off + S) * 64]
                .rearrange("p (s f) -> p s f", f=64))

            # half-select: vd = v_lo + m*(v_hi - v_lo), full-rate DVE
            vd = work.tile([P, S, 64], F16, tag="vd", name="vd" + key,
                           bufs=2)
            nc.vector.tensor_sub(out=vd[:, :, :], in0=v[:, :, 64:128],
                                 in1=v[:, :, 0:64])
            nc.vector.tensor_tensor(out=vd[:, :, :], in0=vd[:, :, :],
                                    in1=mskt[:, :, :], op=ALU.mult)
            nc.vector.tensor_tensor(out=vd[:, :, :], in0=vd[:, :, :],
                                    in1=v[:, :, 0:64], op=ALU.add)

            if len(pend) > 0:
                _emit_pe(pend.pop(0))

            gw = len(sg)
            k = k_sched[key][sg[0]]
            g0 = sg[0]
            if True:
                tiles = sg
                vd4 = vd[:, :, :].rearrange(
                    "p (t k) f -> p t k f", t=gw)

                def vsl(a, b, vd4=vd4):
                    return vd4[:, :, a:b, :]

                v2 = work.tile([P, gw, k, 64], F16, tag="v2",
                               name="v2" + key, bufs=1)
                nc.scalar.activation(out=v2[:, :, :, :], in_=vd4,
                                     func=ACTF.Square)
                v24 = v2[:, :, :, :]

                def vsl2(a, b, v24=v24):
                    return v24[:, :, a:b, :]

                s_ = work.tile([P, gw, 64], F32, tag="s_", name="s_" + key)
                _emit_sum_tree(nc, work, vsl, k, s_[:, :, :], "st")
                q_ = work.tile([P, gw, 64], F32, tag="q_", name="q_" + key)
                _emit_sum_tree(nc, work, vsl2, k, q_[:, :, :], "st")

                # pad compensation (pads replicate slot 0)
                tmp = work.tile([P, gw, 64], F32, tag="tmp",
                                name="tmp" + key)
                gsl = slice(g0, g0 + gw)
                padb = bcast(cols[f"pad_{key}"][:, gsl], gw)
                nc.vector.tensor_tensor(out=tmp[:, :, :],
                                        in0=vd4[:, :, 0, :],
                                        in1=padb, op=ALU.mult)
                nc.vector.tensor_sub(out=s_[:, :, :], in0=s_[:, :, :],
                                     in1=tmp[:, :, :])
                nc.vector.tensor_tensor(out=tmp[:, :, :],
                                        in0=v2[:, :, 0, :],
                                        in1=padb, op=ALU.mult)
                nc.vector.tensor_sub(out=q_[:, :, :], in0=q_[:, :, :],
                                     in1=tmp[:, :, :])

                ag = work.tile([P, gw, 4, 64], F32, tag="aggG" + key,
                               name="aggG" + key)
                idg = bcast(invdegs[key][:, gsl], gw)
                nc.vector.tensor_tensor(out=ag[:, :, 0, :],
                                        in0=s_[:, :, :], in1=idg,
                                        op=ALU.mult)
                nc.vector.tensor_tensor(out=q_[:, :, :], in0=q_[:, :, :],
                                        in1=idg, op=ALU.mult)
                nc.vector.tensor_tensor(out=tmp[:, :, :],
                                        in0=ag[:, :, 0, :],
                                        in1=ag[:, :, 0, :], op=ALU.mult)
                nc.vector.tensor_sub(out=q_[:, :, :], in0=q_[:, :, :],
                                     in1=tmp[:, :, :])
                nc.vector.tensor_scalar_max(out=q_[:, :, :],
                                            in0=q_[:, :, :], scalar1=0.0)
                nc.scalar.activation(out=ag[:, :, 3, :], in_=q_[:, :, :],
                                     func=ACTF.Sqrt, bias=eps_b[:],
                                     scale=1.0)

                _emit_tree(nc, work, vsl, gw, k, ag[:, :, 1, :], ALU.min,
                           "tr")
                _emit_tree(nc, work, vsl, gw, k, ag[:, :, 2, :], ALU.max,
                           "tr")

                # ---- defer PE phase by one supergroup ----
                pend.append((key, tiles, ag, k))

    while pend:
        _emit_pe(pend.pop(0))

    ctx.close()


# --------------------------------------------------------------------------
# Entry point
# --------------------------------------------------------------------------

_CACHE = {}


def make_in_maps(inputs):
    x = np.asarray(inputs["x"], np.float32)
    ei = np.asarray(inputs["edge_index"])
    cores, k_sched, xT = _host_prep(x, ei)
    w = _weights_prep(inputs)
    in_maps = []
    for co in cores:
        m = {"xT_f": xT}
        for key in ("s", "d"):
            m["xperm_" + key] = co["xperm_" + key]
            m["idx_" + key] = co["idx_" + key]
            m["msk_" + key] = co["msk_" + key]
            for nm in COL_NAMES:
                m[f"{nm}_{key}"] = co[f"{nm}_{key}"]
        for nm, shp in WEIGHT_SPECS:
            m[nm] = np.ascontiguousarray(w[nm].reshape(shp))
        in_maps.append(m)
    return cores, k_sched, in_maps


def kernel(**inputs):
    configure(int(np.asarray(inputs["x"]).shape[0]))
    cores, k_sched, in_maps = make_in_maps(inputs)

    key = (CFG.n_nodes, tuple(k_sched["s"]), tuple(k_sched["d"]))
    if key not in _CACHE:
        _CACHE[key] = build_kernel(k_sched)
    nc = _CACHE[key]

    res = bass_utils.run_bass_kernel_spmd(
        nc, in_maps, core_ids=list(range(CFG.n_cores)))

    y_full = np.zeros((CFG.n_nodes, D), np.float32)
    for key in ("s", "d"):
        for c, co in enumerate(cores):
            yc = res.results[c]["y_" + key]
            perm = co["glob_perm_" + key]
            valid = perm >= 0
            y_full[perm[valid]] += yc[valid]
    return y_full


# revision 35
# speedup vs baseline: 1.0540x; 1.0540x over previous
"""DirPNAConv (gnn_message_passing) Trainium2 Bass kernel.

Math: for each direction, messages m_e = cat(x[recv], x[send]) @ preW + preb
split linearly into m_e = A[recv] + B[send] with per-node tables
A = x @ blockdiag(preW[:, :FI]) + preb, B = x @ blockdiag(preW[:, FI:]).
All four PNA aggregators (mean/min/max/std) then reduce to segment
reductions of B[send] over receivers:
  sum S, sumsq Q (A-terms cancel exactly in the variance),
  min/max shift by A[recv].

Sharding: per DIRECTION, nodes are sorted by that direction's degree
and dealt round-robin to the 8 cores. Every core computes the full
B tables locally from a replicated x — no collectives.

The per-edge B rows are fetched with dma_gather (int16 indices into a
pair-packed table btab[r] = [B[r] | B[r + 25088]], 256 B rows); a 3-op
f16 select with a HOST-PRE-EXPANDED mask picks the half. The gather's
Q7 descriptor generation (~7.5 ns/row) is the kernel's hard bottleneck,
so everything is organized to keep the Pool engine 100% busy:
  - per-tile (GS=1) ELL widths -> minimal padding (~2.5%),
  - tiles concatenated into ~16k-row supergroup gathers issued
    back-to-back (v tiles double-buffered; all indices preloaded),
  - the select mask is pre-expanded to [P, S, 64] on the host so the
    select runs at full DVE rate (no free-dim broadcast reads),
  - phase 0 (pair-table build) is deeply pipelined and everything else
    (select, trees, PE phase) hides under the gathers.

Per-tile pads repeat the tile's first slot so min/max are unaffected
and sums subtract padcount*first_slot.
"""

from contextlib import ExitStack

import numpy as np

import concourse.bacc as bacc
import concourse.bass_utils as bass_utils
import concourse.tile as tile
from concourse import bass, mybir
from concourse.masks import make_identity

F32 = mybir.dt.float32
F16 = mybir.dt.float16
I16 = mybir.dt.int16
ACTF = mybir.ActivationFunctionType
ALU = mybir.AluOpType
AXX = mybir.AxisListType.X

P = 128
D, T, FI = 64, 4, 16
AVG_LOG = float(np.log(17.0))
SG_CAP = 64                 # max slot-columns per supergroup gather


class CFG:
    n_nodes = 50000
    n_cores = 8

    @classmethod
    def derived(cls):
        cls.npc = (cls.n_nodes + cls.n_cores - 1) // cls.n_cores
        cls.npc_pad = ((cls.npc + P - 1) // P) * P
        cls.nt = cls.npc_pad // P
        cls.ntot = ((cls.n_nodes + 511) // 512) * 512
        cls.pair_rows = cls.ntot // 2
        cls.groups = [[t] for t in range(cls.nt)]


CFG.derived()


def configure(n_nodes, n_cores=8):
    CFG.n_nodes = n_nodes
    CFG.n_cores = n_cores
    CFG.derived()


def _supergroups(ks):
    """Consecutive equal-k blocks of <= SG_CAP columns (ks is already
    quantized to be constant within each block)."""
    sgs = []
    cur = []
    for t, k in enumerate(ks):
        if cur and (ks[cur[0]] != k or (len(cur) + 1) * k > SG_CAP):
            sgs.append(cur)
            cur = []
        cur.append(t)
    if cur:
        sgs.append(cur)
    return sgs


def _quantize(ks):
    """Pad per-tile widths (non-increasing) up to the first tile of each
    supergroup so every supergroup has one uniform k."""
    out = []
    t = 0
    while t < len(ks):
        k0 = ks[t]
        n = min(max(1, SG_CAP // k0), len(ks) - t)
        out.extend([k0] * n)
        t += n
    return out


# --------------------------------------------------------------------------
# Host-side routing prep (integer index manipulation only, no float math)
# --------------------------------------------------------------------------

def _core_edge_stats(recv, send, members, slot_of_global):
    npp = CFG.npc_pad
    sel = np.isin(recv, members)
    r = recv[sel]
    s = send[sel].astype(np.int64)
    slot = slot_of_global[r]
    order = np.argsort(slot, kind="stable")
    slot, s = slot[order], s[order]
    deg = np.bincount(slot, minlength=npp)
    start = np.zeros(npp, np.int64)
    start[1:] = np.cumsum(deg)[:-1]
    return slot, s, start, deg


def _wrap16(lst):
    assert lst.shape[0] % 16 == 0
    a = lst.astype(np.int16).reshape(-1, 16).T        # [16, S]
    return np.ascontiguousarray(np.tile(a, (8, 1)))   # [128, S]


def _host_prep(x, edge_index):
    src = np.asarray(edge_index[0]).astype(np.int64)
    dst = np.asarray(edge_index[1]).astype(np.int64)
    x = np.asarray(x, np.float32)
    nn, ncores, nt = CFG.n_nodes, CFG.n_cores, CFG.nt
    npp = CFG.npc_pad

    cnt_s2d_g = np.bincount(dst, minlength=nn)
    cnt_d2s_g = np.bincount(src, minlength=nn)
    orders = {"s": np.argsort(-cnt_s2d_g, kind="stable"),
              "d": np.argsort(-cnt_d2s_g, kind="stable")}
    rvsv = {"s": (dst, src), "d": (src, dst)}

    cores = []
    for c in range(ncores):
        co = {}
        for key in ("s", "d"):
            members = orders[key][c::ncores]
            glob_perm = np.full(npp, -1, np.int64)
            glob_perm[:members.shape[0]] = members
            slot_of_global = np.full(nn, -1, np.int64)
            slot_of_global[members] = np.arange(members.shape[0])
            co["glob_perm_" + key] = glob_perm
            rv, sv = rvsv[key]
            co["st_" + key] = _core_edge_stats(rv, sv, members,
                                               slot_of_global)
        cores.append(co)

    # per-tile uniform width = max degree over the tile, all cores;
    # then quantized so each supergroup gets one uniform k
    k_sched = {}
    for key in ("s", "d"):
        ks = []
        for t in range(nt):
            g0, g1 = t * P, (t + 1) * P
            kmax = 2
            for co in cores:
                _, _, _, deg = co["st_" + key]
                kmax = max(kmax, int(deg[g0:g1].max()))
            ks.append(kmax)
        k_sched[key] = _quantize(ks)

    for co in cores:
        for key in ("s", "d"):
            slot, s, start, deg = co.pop("st_" + key)
            kmax = max(k_sched[key])
            ell = np.full((npp, kmax), -1, np.int64)
            pos = np.arange(s.shape[0], dtype=np.int64) - start[slot]
            ell[slot, pos] = s
            first = ell[:, 0].copy()
            first[first < 0] = 0
            m = ell < 0
            ell[m] = np.broadcast_to(first[:, None], ell.shape)[m]
            idx_chunks, msk_chunks = [], []
            for t in range(nt):
                k = k_sched[key][t]
                blk = ell[t * P:(t + 1) * P, :k]          # [P, k]
                half = CFG.pair_rows
                idx_chunks.append(np.ascontiguousarray(
                    (blk % half).T).reshape(-1))          # [k, P] flat
                msk_chunks.append((blk >= half).T)        # [k, P]
            co["idx_" + key] = _wrap16(np.concatenate(idx_chunks))
            # expanded select mask: [P, Stot, 64] -> [P, Stot*64] f16
            mk = np.concatenate(msk_chunks, axis=0)       # [Stot, P]
            mke = np.repeat(mk.T.astype(np.float16)[:, :, None], 64,
                            axis=2)
            co["msk_" + key] = np.ascontiguousarray(
                mke.reshape(P, -1))                       # [128, Stot*64]
            degc = np.maximum(deg, 1).astype(np.float32)
            co["deg_" + key] = np.ascontiguousarray(
                degc.reshape(nt, P).T).astype(np.float32)     # [128, nt]
            sch = np.asarray(k_sched[key], np.int64)
            padc = (sch[:, None] - deg.reshape(nt, P)).T.astype(np.float32)
            co["pad_" + key] = np.ascontiguousarray(padc)     # [128, nt]
            xp = np.zeros((npp, D), np.float32)
            valid = co["glob_perm_" + key] >= 0
            xp[valid] = x[co["glob_perm_" + key][valid]]
            co["xperm_" + key] = xp

    xT = np.zeros((D, CFG.ntot), np.float16)
    xT[:, :nn] = x.T.astype(np.float16)
    return cores, k_sched, xT


def _blockdiag(w):  # w: [T, FI, FO] -> [T*FI, T*FO]
    t, fi, fo = w.shape
    out = np.zeros((t * fi, t * fo), np.float32)
    for i in range(t):
        out[i * fi:(i + 1) * fi, i * fo:(i + 1) * fo] = w[i]
    return out


def _weights_prep(inp):
    """Pure re-layout of the input weights (no arithmetic)."""
    w = {}
    for dk in ("s2d", "d2s"):
        preW = np.asarray(inp["pre_W_" + dk], np.float32)   # [T, 2FI, FI]
        preb = np.asarray(inp["pre_b_" + dk], np.float32).reshape(-1)  # [64]
        WA = _blockdiag(preW[:, :FI, :])                    # [64, 64]
        WB = _blockdiag(preW[:, FI:, :])                    # [64, 64]
        dup = np.zeros((65, 128), np.float32)
        dup[:64, :64] = WA
        dup[:64, 64:] = WA
        dup[64, :64] = preb
        dup[64, 64:] = preb
        half = np.zeros((65, 128), np.float32)
        half[:64, :64] = WA
        half[64, :64] = preb
        w["WAdup_" + dk] = dup
        w["WAhalf_" + dk] = half
        w["WB_" + dk] = WB
        postW = np.asarray(inp["post_W_" + dk], np.float32)  # [T, 208, 16]
        P0 = _blockdiag(postW[:, 0:FI, :])                   # [64, 64]
        Ps = []
        for blk in range(3):                                 # 1, amp, 1/amp
            Pg = np.zeros((256, 64), np.float32)
            for a in range(4):                               # mean/mn/mx/std
                for t in range(T):
                    rows = FI + blk * 4 * FI + a * FI
                    Pg[a * 64 + t * FI:a * 64 + (t + 1) * FI,
                       t * FI:(t + 1) * FI] = postW[t, rows:rows + FI, :]
            Ps.append(Pg)
        w["P0T_" + dk] = np.ascontiguousarray(P0.T)          # [64, 64]
        for i, Pg in enumerate(Ps):
            w[f"P{i+1}T_{dk}"] = np.ascontiguousarray(Pg.T)  # [64, 256]
        w["linW_" + dk] = np.asarray(inp["lin_W_" + dk], np.float32)
        w["linb_" + dk] = np.asarray(
            inp["lin_b_" + dk], np.float32).reshape(1, 64)
        w["postb_col_" + dk] = np.asarray(
            inp["post_b_" + dk], np.float32).reshape(64, 1)
    wbp = np.zeros((64, 128), np.float32)
    wbp[:, :64] = w["WB_s2d"]
    wbp[:, 64:] = w["WB_d2s"]
    w["WBpair"] = wbp
    w["selfW"] = np.asarray(inp["lin_self_W"], np.float32)
    w["selfb"] = np.asarray(inp["lin_self_b"], np.float32).reshape(1, 64)
    w["alpha"] = np.asarray(inp["alpha"], np.float32).reshape(1, 1)
    return w


# --------------------------------------------------------------------------
# Device kernel
# --------------------------------------------------------------------------

WEIGHT_SPECS = [
    ("WBpair", (64, 128)),
    ("WAdup_s2d", (65, 128)), ("WAdup_d2s", (65, 128)),
    ("WAhalf_s2d", (65, 128)), ("WAhalf_d2s", (65, 128)),
    ("P0T_s2d", (64, 64)), ("P0T_d2s", (64, 64)),
    ("P1T_s2d", (64, 256)), ("P1T_d2s", (64, 256)),
    ("P2T_s2d", (64, 256)), ("P2T_d2s", (64, 256)),
    ("P3T_s2d", (64, 256)), ("P3T_d2s", (64, 256)),
    ("linW_s2d", (64, 64)), ("linW_d2s", (64, 64)),
    ("linb_s2d", (1, 64)), ("linb_d2s", (1, 64)),
    ("postb_col_s2d", (64, 1)), ("postb_col_d2s", (64, 1)),
    ("selfW", (64, 64)), ("selfb", (1, 64)),
    ("alpha", (1, 1)),
]
COL_NAMES = ["deg", "pad"]


def _emit_tree(nc, pool, vsl, gw, k, out_f32, op, tag):
    """Run-wide min/max tree over vsl(a, b) -> AP [128, gw, b-a, 64]
    (f16). Overlap-pairing (idempotent ops) avoids odd-element carries."""
    if k == 1:
        nc.vector.tensor_copy(out=out_f32, in_=vsl(0, 1)[:, :, 0, :])
        return
    if k == 2:
        nc.vector.tensor_tensor(out=out_f32, in0=vsl(0, 1)[:, :, 0, :],
                                in1=vsl(1, 2)[:, :, 0, :], op=op)
        return
    h = (k + 1) // 2
    tmp = pool.tile([P, gw, max(2, (k + 1) // 2), 64], F16, tag=tag,
                    name=tag, bufs=1)
    nc.vector.tensor_tensor(out=tmp[:, :, :h, :], in0=vsl(0, h),
                            in1=vsl(k - h, k), op=op)
    m = h
    while m > 2:
        h = (m + 1) // 2
        nc.vector.tensor_tensor(out=tmp[:, :, :h, :], in0=tmp[:, :, :h, :],
                                in1=tmp[:, :, m - h:m, :], op=op)
        m = h
    nc.vector.tensor_tensor(out=out_f32, in0=tmp[:, :, 0, :],
                            in1=tmp[:, :, 1, :], op=op)


def _emit_sum_tree(nc, pool, first_in, k, out_f32, tag):
    """Run-wide exact sum tree over first_in(a, b) -> [P, gw, b-a, 64]
    (f16 source)."""
    gw = out_f32.shape[1]
    if k == 2:
        nc.vector.tensor_tensor(out=out_f32, in0=first_in(0, 1)[:, :, 0, :],
                                in1=first_in(1, 2)[:, :, 0, :], op=ALU.add)
        return
    if k == 3:
        nc.vector.tensor_tensor(out=out_f32, in0=first_in(0, 1)[:, :, 0, :],
                                in1=first_in(1, 2)[:, :, 0, :], op=ALU.add)
        nc.vector.tensor_tensor(out=out_f32, in0=out_f32,
                                in1=first_in(2, 3)[:, :, 0, :], op=ALU.add)
        return
    m = k // 2
    tmpb = pool.tile([P, gw, m, 64], F32, tag=tag, name=tag, bufs=1)
    nc.vector.tensor_tensor(out=tmpb[:, :, :m, :], in0=first_in(0, m),
                            in1=first_in(m, 2 * m), op=ALU.add)
    while m > 2:
        h, odd = m // 2, m % 2
        nc.vector.tensor_tensor(out=tmpb[:, :, :h, :], in0=tmpb[:, :, :h, :],
                                in1=tmpb[:, :, h + odd:m, :], op=ALU.add)
        m = h + odd
    nc.vector.tensor_tensor(out=out_f32, in0=tmpb[:, :, 0, :],
                            in1=tmpb[:, :, 1, :], op=ALU.add)
    if k % 2:
        nc.vector.tensor_tensor(out=out_f32, in0=out_f32,
                                in1=first_in(k - 1, k)[:, :, 0, :],
                                op=ALU.add)


def build_kernel(k_sched):
    nt, ntot, npc_pad = CFG.nt, CFG.ntot, CFG.npc_pad
    nc = bacc.Bacc("TRN2", target_bir_lowering=False, debug=False,
                   num_devices=CFG.n_cores)

    din = {}
    din["xT_f"] = nc.dram_tensor("xT_f", [64, ntot], F16,
                                 kind="ExternalInput").ap()
    y_dram = {}
    for key in ("s", "d"):
        din["xperm_" + key] = nc.dram_tensor(
            "xperm_" + key, [npc_pad, 64], F32, kind="ExternalInput").ap()
        stot = sum(k_sched[key])
        din["idx_" + key] = nc.dram_tensor(
            "idx_" + key, [P, 8 * stot], I16, kind="ExternalInput").ap()
        din["msk_" + key] = nc.dram_tensor(
            "msk_" + key, [P, stot * 64], F16, kind="ExternalInput").ap()
        for nm in COL_NAMES:
            din[f"{nm}_{key}"] = nc.dram_tensor(
                f"{nm}_{key}", [P, nt], F32, kind="ExternalInput").ap()
        y_dram[key] = nc.dram_tensor("y_" + key, [npc_pad, 64], F32,
                                     kind="ExternalOutput").ap()
    for nm, shp in WEIGHT_SPECS:
        din[nm] = nc.dram_tensor(nm, list(shp), F32,
                                 kind="ExternalInput").ap()
    btab = {
        "s": nc.dram_tensor("btab_s", [CFG.pair_rows, 128], F16,
                            kind="Internal").ap(),
        "d": nc.dram_tensor("btab_d", [CFG.pair_rows, 128], F16,
                            kind="Internal").ap(),
    }

    with tile.TileContext(nc) as tc:
        _emit(tc, nc, din, y_dram, btab, k_sched)

    nc.compile()
    return nc


def _emit(tc, nc, din, y_dram, btab, k_sched):
    nt, ntot = CFG.nt, CFG.ntot
    ctx = ExitStack()
    consts = ctx.enter_context(tc.tile_pool(name="consts", bufs=1))
    small = ctx.enter_context(tc.tile_pool(name="small", bufs=3))
    work = ctx.enter_context(tc.tile_pool(name="work", bufs=2))

    # ---- constants ------------------------------------------------------
    ident = consts.tile([P, P], F32)
    make_identity(nc, ident[:])
    eps_b = consts.tile([P, 1], F32)
    nc.vector.memset(eps_b[:], 1e-5)

    w_sb = {}
    for nm, shp in WEIGHT_SPECS:
        t = consts.tile([shp[0], shp[1]], F32, tag="w_" + nm)
        nc.sync.dma_start(out=t[:], in_=din[nm][:, :])
        w_sb[nm] = t

    # preload ALL gather indices (both directions) once
    idx_sb = {}
    for key in ("s", "d"):
        stot = sum(k_sched[key])
        it = consts.tile([P, 8 * stot], I16, tag="idx" + key,
                         name="idx" + key)
        nc.sync.dma_start(out=it[:], in_=din["idx_" + key][:, :])
        idx_sb[key] = it

    cols = {}
    amps, invamps, invdegs = {}, {}, {}
    for key in ("s", "d"):
        for nm in COL_NAMES:
            cname = f"{nm}_{key}"
            ct = consts.tile([P, nt], F32, tag=cname, name=cname)
            nc.sync.dma_start(out=ct[:], in_=din[cname][:, :])
            cols[cname] = ct
        amps[key] = consts.tile([P, nt], F32, tag="amp" + key,
                                name="amp" + key)
        nc.scalar.activation(out=amps[key][:], in_=cols["deg_" + key][:],
                             func=ACTF.Ln, bias=1.0, scale=1.0)
        invamps[key] = consts.tile([P, nt], F32, tag="iamp" + key,
                                   name="iamp" + key)
        nc.vector.reciprocal(out=invamps[key][:], in_=amps[key][:])
        invdegs[key] = consts.tile([P, nt], F32, tag="ideg" + key,
                                   name="ideg" + key)
        nc.vector.reciprocal(out=invdegs[key][:], in_=cols["deg_" + key][:])

    # ---- alpha, scaled linW, G matrices, bias ---------------------------
    alpha_b = consts.tile([64, 1], F32)
    nc.gpsimd.dma_start(
        out=alpha_b[:],
        in_=bass.AP(tensor=din["alpha"].tensor, offset=0,
                    ap=[[0, 64], [1, 1]]))
    a_d2s = alpha_b
    a_s2d = consts.tile([64, 1], F32)
    nc.vector.memset(a_s2d[:], 1.0)
    nc.vector.tensor_sub(out=a_s2d[:], in0=a_s2d[:], in1=alpha_b[:])

    alph = {"s": a_s2d, "d": a_d2s}
    dk_of = {"s": "s2d", "d": "d2s"}
    linWs = {}
    for key in ("s", "d"):
        lw = consts.tile([64, 64], F32, tag="linWs" + key, name="linWs" + key)
        nc.vector.tensor_scalar_mul(
            out=lw[:], in0=w_sb["linW_" + dk_of[key]][:], scalar1=alph[key][:])
        linWs[key] = lw

    G = {}
    G0 = {}
    selfW_ext = consts.tile([65, 64], F32)
    nc.sync.dma_start(out=selfW_ext[:64, :], in_=din["selfW"][:, :])

    wbpair16 = consts.tile([64, 128], F16, tag="wbpair16")
    nc.vector.tensor_copy(out=wbpair16[:], in_=w_sb["WBpair"][:])

    scale_of = {1: 1.0, 2: 1.0 / AVG_LOG, 3: AVG_LOG}
    with tc.tile_pool(name="setup_ps", bufs=4, space="PSUM") as setup_ps:
        # ---- phase 0: pair-packed B tables ------------------------------
        # btab_<dir>[r, :] = [B[r] | B[r + 25088]]; a 1024-node chunk
        # writes node-major with contiguous 128 B runs. Loads on sync,
        # matmuls on PE, f16 casts + table writes on ACT.
        CH = 1024
        n_chunks = ntot // CH
        half = CFG.pair_rows
        ph0_cm = tc.tile_pool(name="ph0", bufs=1)
        ph0 = ph0_cm.__enter__()
        # build btab_s for ALL nodes first (s-gathers can then start),
        # then btab_d underneath the early s-gathers.
        for key, p0 in (("s", 0), ("d", 64)):
            for ci in range(n_chunks):
                xch = ph0.tile([64, CH], F16, tag="xch", bufs=3)
                nc.scalar.dma_start(out=xch[:],
                                    in_=din["xT_f"][:, ci * CH:(ci + 1) * CH])
                ps_big = setup_ps.tile([P, CH // 2], F32, tag="bps",
                                       name="bps", bufs=2)
                for j in range(CH // P):
                    nc.tensor.matmul(out=ps_big[:, j * 64:(j + 1) * 64],
                                     lhsT=xch[:, j * P:(j + 1) * P],
                                     rhs=wbpair16[:, p0:p0 + 64],
                                     start=True, stop=True)
                nj = CH // P
                bsb = ph0.tile([P, nj, 64], F16, tag="bsb", bufs=3)
                nc.scalar.copy(
                    out=bsb[:, :, :],
                    in_=ps_big[:].rearrange("p (j c) -> p j c", j=nj))
                pieces = []
                n0 = ci * CH
                jmid = (half - n0) // P
                if jmid <= 0:
                    pieces.append((0, nj, n0 - half, 64))
                elif jmid >= nj:
                    pieces.append((0, nj, n0, 0))
                else:
                    pieces.append((0, jmid, n0, 0))
                    pieces.append((jmid, nj, n0 + jmid * P - half, 64))
                for j0, j1, row0, c0 in pieces:
                    out_ap = bass.AP(tensor=btab[key].tensor,
                                     offset=row0 * 128 + c0,
                                     ap=[[128, 128], [128 * 128, j1 - j0],
                                         [1, 64]])
                    nc.sync.dma_start(out=out_ap,
                                       in_=bsb[:, j0:j1, :])
        ph0_cm.__exit__(None, None, None)
        for key in ("s", "d"):
            dk = dk_of[key]
            for i in (1, 2, 3):
                for c in (0, 1):
                    ps = setup_ps.tile([P, 64], F32, tag="gps", name="gps",
                                       bufs=1)
                    nc.tensor.matmul(
                        out=ps[:],
                        lhsT=w_sb[f"P{i}T_{dk}"][:, c * P:(c + 1) * P],
                        rhs=linWs[key][:], start=True, stop=True)
                    g = consts.tile([P, 64], F32, tag=f"G{i}{key}{c}",
                                    name=f"G{i}{key}{c}")
                    nc.scalar.activation(out=g[:], in_=ps[:], func=ACTF.Copy,
                                         scale=scale_of[i])
                    G[f"{i}{key}{c}"] = g
            ps = setup_ps.tile([64, 64], F32, tag="g0ps", name="g0ps",
                             bufs=1)
            nc.tensor.matmul(out=ps[:], lhsT=w_sb[f"P0T_{dk}"][:],
                             rhs=linWs[key][:], start=True, stop=True)
            g0 = consts.tile([P, 64], F32, tag="G0" + key, name="G0" + key)
            nc.vector.tensor_copy(out=g0[:64, :], in_=ps[:])
            G0[key] = g0

        bias_ps = setup_ps.tile([1, 64], F32, tag="biasps",
                                name="biasps", bufs=1)
        nc.tensor.matmul(out=bias_ps[:], lhsT=w_sb["postb_col_s2d"][:],
                         rhs=linWs["s"][:], start=True, stop=False)
        nc.tensor.matmul(out=bias_ps[:], lhsT=w_sb["postb_col_d2s"][:],
                         rhs=linWs["d"][:], start=False, stop=True)
        tb = small.tile([1, 64], F32, tag="tb")
        nc.vector.tensor_scalar_mul(out=tb[:], in0=w_sb["linb_s2d"][:],
                                    scalar1=a_s2d[:1, :])
        nc.vector.tensor_add(out=tb[:], in0=tb[:], in1=bias_ps[:])
        tb2 = small.tile([1, 64], F32, tag="tb2")
        nc.vector.tensor_scalar_mul(out=tb2[:], in0=w_sb["linb_d2s"][:],
                                    scalar1=a_d2s[:1, :])
        nc.vector.tensor_add(out=tb[:], in0=tb[:], in1=tb2[:])
        nc.vector.tensor_add(out=tb[:], in0=tb[:], in1=w_sb["selfb"][:])
        nc.sync.dma_start(out=selfW_ext[64:65, :], in_=tb[:])

    # ---- main loop ------------------------------------------------------
    psum = ctx.enter_context(tc.tile_pool(name="psum", bufs=1, space="PSUM"))


    def bcast(col_ap, gw):
        # [128, gw] column slice -> [128, gw, 64] free-broadcast AP
        return col_ap.unsqueeze(2).to_broadcast([P, gw, 64])

    pend = []

    def _emit_pe(item):
        key, tiles, ag, k = item
        for ti, t in enumerate(tiles):
            xp = small.tile([P, 64], F32, tag="xp")
            nc.sync.dma_start(
                out=xp[:],
                in_=din["xperm_" + key][t * P:(t + 1) * P, :])
            xpT_ps = psum.tile([64, P], F32, tag="tp",
                               name="xpT_ps", bufs=2)
            nc.tensor.transpose(out=xpT_ps[:], in_=xp[:],
                                identity=ident[:])
            xpT32 = small.tile([65, P], F32, tag="xpT32")
            nc.scalar.copy(out=xpT32[:64, :], in_=xpT_ps[:])
            nc.vector.memset(xpT32[64:65, :], 1.0)

            u1 = psum.tile([64, P], F32, tag="u1", name="u1" + key,
                           bufs=2)
            aggT = work.tile([P, 2, P], F32, tag="aggT",
                             name="aggT" + key)
            for c, wkind in ((0, "dup"), (1, "half")):
                tp = psum.tile([P, P], F32, tag="tp", name="tp",
                               bufs=2)
                nc.tensor.matmul(out=tp[:],
                                 lhsT=ag[:, ti, 2 * c:2 * c + 2, :],
                                 rhs=ident[:], is_transpose=True,
                                 start=True, stop=False,
                                 skip_group_check=True)
                nc.tensor.matmul(
                    out=tp[:],
                    lhsT=w_sb[f"WA{wkind}_{dk_of[key]}"][:],
                    rhs=xpT32[:], start=False, stop=True,
                    skip_group_check=True)
                nc.scalar.copy(out=aggT[:, c, :], in_=tp[:])

            nc.tensor.matmul(out=u1[:], lhsT=G[f"1{key}0"][:],
                             rhs=aggT[:, 0, :], start=True,
                             stop=False, skip_group_check=True)
            nc.tensor.matmul(out=u1[:], lhsT=G[f"1{key}1"][:],
                             rhs=aggT[:, 1, :], start=False,
                             stop=False, skip_group_check=True)
            nc.tensor.matmul(out=u1[:], lhsT=G0[key][:64, :],
                             rhs=xpT32[:64, :], start=False,
                             stop=(key == "d"),
                             skip_group_check=True)
            if key == "s":
                nc.tensor.matmul(out=u1[:], lhsT=selfW_ext[:],
                                 rhs=xpT32[:], start=False,
                                 stop=True, skip_group_check=True)
            u23 = psum.tile([P, P], F32, tag="u23",
                            name="u23" + key, bufs=2)
            nc.tensor.matmul(out=u23[:64, :], lhsT=G[f"2{key}0"][:],
                             rhs=aggT[:, 0, :], start=True,
                             stop=False, skip_group_check=True)
            nc.tensor.matmul(out=u23[:64, :], lhsT=G[f"2{key}1"][:],
                             rhs=aggT[:, 1, :], start=False,
                             stop=True, skip_group_check=True)
            nc.tensor.matmul(out=u23[64:, :], lhsT=G[f"3{key}0"][:],
                             rhs=aggT[:, 0, :], start=True,
                             stop=False, skip_group_check=True)
            nc.tensor.matmul(out=u23[64:, :], lhsT=G[f"3{key}1"][:],
                             rhs=aggT[:, 1, :], start=False,
                             stop=True, skip_group_check=True)

            u1sb = small.tile([64, P], F32, tag="u1sb")
            nc.scalar.copy(out=u1sb[:], in_=u1[:])
            u1T = psum.tile([P, 64], F32, tag="utr", name="u1T",
                            bufs=2)
            nc.tensor.transpose(out=u1T[:], in_=u1sb[:],
                                identity=ident[:64, :64])
            upk = small.tile([P, P], F32, tag="upk",
                             name="upk" + key)
            nc.scalar.copy(out=upk[:], in_=u23[:])
            uT = psum.tile([P, P], F32, tag="utr", name="uT" + key,
                           bufs=2)
            nc.tensor.transpose(out=uT[:], in_=upk[:],
                                identity=ident[:])
            y_sb = small.tile([P, 64], F32, tag="y_sb")
            sc = small.tile([P, 64], F32, tag="sc", name="sc" + key)
            nc.scalar.activation(out=sc[:], in_=uT[:, 0:64],
                                 func=ACTF.Copy,
                                 scale=amps[key][:, t:t + 1])
            nc.vector.tensor_add(out=y_sb[:], in0=u1T[:], in1=sc[:])
            nc.scalar.activation(out=sc[:], in_=uT[:, 64:128],
                                 func=ACTF.Copy,
                                 scale=invamps[key][:, t:t + 1])
            nc.vector.tensor_add(out=y_sb[:], in0=y_sb[:],
                                 in1=sc[:])
            nc.sync.dma_start(
                out=y_dram[key][t * P:(t + 1) * P, :], in_=y_sb[:])

    for key in ("s", "d"):
        sgs = _supergroups(k_sched[key])
        offs = []
        o = 0
        for sg in sgs:
            offs.append(o)
            o += sum(k_sched[key][t] for t in sg)
        # descending-k pipeline, but finish with the few-tile head groups
        # so the post-gather tail (trees + PE phase) is short
        order = list(zip(sgs, offs))
        order = order[2:] + order[:2][::-1]
        for sg, off in order:
            S = sum(k_sched[key][t] for t in sg)
            v = work.tile([P, S, 128], F16, tag="vg", name="vg" + key,
                          bufs=3)
            nc.gpsimd.dma_gather(
                out_ap=v[:, :, :], in_ap=btab[key][:, :],
                idxs_ap=idx_sb[key][:, 8 * off:8 * (off + S)],
                num_idxs=P * S, num_idxs_reg=P * S, elem_size=128,
                single_packet=False)

            mskt = work.tile([P, S, 64], F16, tag="msk", name="msk" + key,
                             bufs=3)
            nc.sync.dma_start(
                out=mskt[:],
                in_=din["msk_" + key][:, off * 64:(off + S) * 64]
                .rearrange("p (s f) -> p s f", f=64))

            # half-select: vd = v_lo + m*(v_hi - v_lo), full-rate DVE
            vd = work.tile([P, S, 64], F16, tag="vd", name="vd" + key,
                           bufs=2)
            nc.vector.tensor_sub(out=vd[:, :, :], in0=v[:, :, 64:128],
                                 in1=v[:, :, 0:64])
            nc.vector.tensor_tensor(out=vd[:, :, :], in0=vd[:, :, :],
                                    in1=mskt[:, :, :], op=ALU.mult)
            nc.vector.tensor_tensor(out=vd[:, :, :], in0=vd[:, :, :],
                                    in1=v[:, :, 0:64], op=ALU.add)

            if len(pend) > 0:
                _emit_pe(pend.pop(0))

            gw = len(sg)
            k = k_sched[key][sg[0]]
            g0 = sg[0]
            if True:
                tiles = sg
                vd4 = vd[:, :, :].rearrange(
                    "p (t k) f -> p t k f", t=gw)

                def vsl(a, b, vd4=vd4):
                    return vd4[:, :, a:b, :]

                v2 = work.tile([P, gw, k, 64], F16, tag="v2",
                               name="v2" + key, bufs=1)
                nc.scalar.activation(out=v2[:, :, :, :], in_=vd4,
                                     func=ACTF.Square)
                v24 = v2[:, :, :, :]

                def vsl2(a, b, v24=v24):
                    return v24[:, :, a:b, :]

                s_ = work.tile([P, gw, 64], F32, tag="s_", name="s_" + key)
                _emit_sum_tree(nc, work, vsl, k, s_[:, :, :], "st")
                q_ = work.tile([P, gw, 64], F32, tag="q_", name="q_" + key)
                _emit_sum_tree(nc, work, vsl2, k, q_[:, :, :], "st")

                # pad compensation (pads replicate slot 0)
                tmp = work.tile([P, gw, 64], F32, tag="tmp",
                                name="tmp" + key)
                gsl = slice(g0, g0 + gw)
                padb = bcast(cols[f"pad_{key}"][:, gsl], gw)
                nc.vector.tensor_tensor(out=tmp[:, :, :],
                                        in0=vd4[:, :, 0, :],
                                        in1=padb, op=ALU.mult)
                nc.vector.tensor_sub(out=s_[:, :, :], in0=s_[:, :, :],
                                     in1=tmp[:, :, :])
                nc.vector.tensor_tensor(out=tmp[:, :, :],
                                        in0=v2[:, :, 0, :],
                                        in1=padb, op=ALU.mult)
                nc.vector.tensor_sub(out=q_[:, :, :], in0=q_[:, :, :],
                                     in1=tmp[:, :, :])

                ag = work.tile([P, gw, 4, 64], F32, tag="aggG" + key,
                               name="aggG" + key)
                idg = bcast(invdegs[key][:, gsl], gw)
                nc.vector.tensor_tensor(out=ag[:, :, 0, :],
                                        in0=s_[:, :, :], in1=idg,
                                        op=ALU.mult)
                nc.vector.tensor_tensor(out=q_[:, :, :], in0=q_[:, :, :],
                                        in1=idg, op=ALU.mult)
                nc.vector.tensor_tensor(out=tmp[:, :, :],
                                        in0=ag[:, :, 0, :],
                                        in1=ag[:, :, 0, :], op=ALU.mult)
                nc.vector.tensor_sub(out=q_[:, :, :], in0=q_[:, :, :],
                                     in1=tmp[:, :, :])
                nc.vector.tensor_scalar_max(out=q_[:, :, :],
                                            in0=q_[:, :, :], scalar1=0.0)
                nc.scalar.activation(out=ag[:, :, 3, :], in_=q_[:, :, :],
                                     func=ACTF.Sqrt, bias=eps_b[:],
                                     scale=1.0)

                _emit_tree(nc, work, vsl, gw, k, ag[:, :, 1, :], ALU.min,
                           "tr")
                _emit_tree(nc, work, vsl, gw, k, ag[:, :, 2, :], ALU.max,
                           "tr")

                # ---- defer PE phase by one supergroup ----
                pend.append((key, tiles, ag, k))

    while pend:
        _emit_pe(pend.pop(0))

    ctx.close()


# --------------------------------------------------------------------------
# Entry point
# --------------------------------------------------------------------------

_CACHE = {}


def make_in_maps(inputs):
    x = np.asarray(inputs["x"], np.float32)
    ei = np.asarray(inputs["edge_index"])
    cores, k_sched, xT = _host_prep(x, ei)
    w = _weights_prep(inputs)
    in_maps = []
    for co in cores:
        m = {"xT_f": xT}
        for key in ("s", "d"):
            m["xperm_" + key] = co["xperm_" + key]
            m["idx_" + key] = co["idx_" + key]
            m["msk_" + key] = co["msk_" + key]
            for nm in COL_NAMES:
                m[f"{nm}_{key}"] = co[f"{nm}_{key}"]
        for nm, shp in WEIGHT_SPECS:
            m[nm] = np.ascontiguousarray(w[nm].reshape(shp))
        in_maps.append(m)
    return cores, k_sched, in_maps


def kernel(**inputs):
    configure(int(np.asarray(inputs["x"]).shape[0]))
    cores, k_sched, in_maps = make_in_maps(inputs)

    key = (CFG.n_nodes, tuple(k_sched["s"]), tuple(k_sched["d"]))
    if key not in _CACHE:
        _CACHE[key] = build_kernel(k_sched)
    nc = _CACHE[key]

    res = bass_utils.run_bass_kernel_spmd(
        nc, in_maps, core_ids=list(range(CFG.n_cores)))

    y_full = np.zeros((CFG.n_nodes, D), np.float32)
    for key in ("s", "d"):
        for c, co in enumerate(cores):
            yc = res.results[c]["y_" + key]
            perm = co["glob_perm_" + key]
            valid = perm >= 0
            y_full[perm[valid]] += yc[valid]
    return y_full


# revision 36
# speedup vs baseline: 1.0557x; 1.0015x over previous
"""DirPNAConv (gnn_message_passing) Trainium2 Bass kernel.

Math: for each direction, messages m_e = cat(x[recv], x[send]) @ preW + preb
split linearly into m_e = A[recv] + B[send] with per-node tables
A = x @ blockdiag(preW[:, :FI]) + preb, B = x @ blockdiag(preW[:, FI:]).
All four PNA aggregators (mean/min/max/std) then reduce to segment
reductions of B[send] over receivers:
  sum S, sumsq Q (A-terms cancel exactly in the variance),
  min/max shift by A[recv].

Sharding: per DIRECTION, nodes are sorted by that direction's degree
and dealt round-robin to the 8 cores. Every core computes the full
B tables locally from a replicated x — no collectives.

The per-edge B rows are fetched with dma_gather (int16 indices into a
pair-packed table btab[r] = [B[r] | B[r + 25088]], 256 B rows); a 3-op
f16 select with a HOST-PRE-EXPANDED mask picks the half. The gather's
Q7 descriptor generation (~7.5 ns/row) is the kernel's hard bottleneck,
so everything is organized to keep the Pool engine 100% busy:
  - per-tile (GS=1) ELL widths -> minimal padding (~2.5%),
  - tiles concatenated into ~16k-row supergroup gathers issued
    back-to-back (v tiles double-buffered; all indices preloaded),
  - the select mask is pre-expanded to [P, S, 64] on the host so the
    select runs at full DVE rate (no free-dim broadcast reads),
  - phase 0 (pair-table build) is deeply pipelined and everything else
    (select, trees, PE phase) hides under the gathers.

Per-tile pads repeat the tile's first slot so min/max are unaffected
and sums subtract padcount*first_slot.
"""

from contextlib import ExitStack

import numpy as np

import concourse.bacc as bacc
import concourse.bass_utils as bass_utils
import concourse.tile as tile
from concourse import bass, mybir
from concourse.masks import make_identity

F32 = mybir.dt.float32
F16 = mybir.dt.float16
I16 = mybir.dt.int16
ACTF = mybir.ActivationFunctionType
ALU = mybir.AluOpType
AXX = mybir.AxisListType.X

P = 128
D, T, FI = 64, 4, 16
AVG_LOG = float(np.log(17.0))
SG_CAP = 64                 # max slot-columns per supergroup gather


class CFG:
    n_nodes = 50000
    n_cores = 8

    @classmethod
    def derived(cls):
        cls.npc = (cls.n_nodes + cls.n_cores - 1) // cls.n_cores
        cls.npc_pad = ((cls.npc + P - 1) // P) * P
        cls.nt = cls.npc_pad // P
        cls.ntot = ((cls.n_nodes + 511) // 512) * 512
        cls.pair_rows = cls.ntot // 2
        cls.groups = [[t] for t in range(cls.nt)]


CFG.derived()


def configure(n_nodes, n_cores=8):
    CFG.n_nodes = n_nodes
    CFG.n_cores = n_cores
    CFG.derived()


def _supergroups(ks):
    """Consecutive equal-k blocks of <= SG_CAP columns (ks is already
    quantized to be constant within each block)."""
    sgs = []
    cur = []
    for t, k in enumerate(ks):
        if cur and (ks[cur[0]] != k or (len(cur) + 1) * k > SG_CAP):
            sgs.append(cur)
            cur = []
        cur.append(t)
    if cur:
        sgs.append(cur)
    return sgs


def _quantize(ks):
    """Pad per-tile widths (non-increasing) up to the first tile of each
    supergroup so every supergroup has one uniform k."""
    out = []
    t = 0
    while t < len(ks):
        k0 = ks[t]
        n = min(max(1, SG_CAP // k0), len(ks) - t)
        out.extend([k0] * n)
        t += n
    return out


# --------------------------------------------------------------------------
# Host-side routing prep (integer index manipulation only, no float math)
# --------------------------------------------------------------------------

def _core_edge_stats(recv, send, members, slot_of_global):
    npp = CFG.npc_pad
    sel = np.isin(recv, members)
    r = recv[sel]
    s = send[sel].astype(np.int64)
    slot = slot_of_global[r]
    order = np.argsort(slot, kind="stable")
    slot, s = slot[order], s[order]
    deg = np.bincount(slot, minlength=npp)
    start = np.zeros(npp, np.int64)
    start[1:] = np.cumsum(deg)[:-1]
    return slot, s, start, deg


def _wrap16(lst):
    assert lst.shape[0] % 16 == 0
    a = lst.astype(np.int16).reshape(-1, 16).T        # [16, S]
    return np.ascontiguousarray(np.tile(a, (8, 1)))   # [128, S]


def _host_prep(x, edge_index):
    src = np.asarray(edge_index[0]).astype(np.int64)
    dst = np.asarray(edge_index[1]).astype(np.int64)
    x = np.asarray(x, np.float32)
    nn, ncores, nt = CFG.n_nodes, CFG.n_cores, CFG.nt
    npp = CFG.npc_pad

    cnt_s2d_g = np.bincount(dst, minlength=nn)
    cnt_d2s_g = np.bincount(src, minlength=nn)
    orders = {"s": np.argsort(-cnt_s2d_g, kind="stable"),
              "d": np.argsort(-cnt_d2s_g, kind="stable")}
    rvsv = {"s": (dst, src), "d": (src, dst)}

    cores = []
    for c in range(ncores):
        co = {}
        for key in ("s", "d"):
            members = orders[key][c::ncores]
            glob_perm = np.full(npp, -1, np.int64)
            glob_perm[:members.shape[0]] = members
            slot_of_global = np.full(nn, -1, np.int64)
            slot_of_global[members] = np.arange(members.shape[0])
            co["glob_perm_" + key] = glob_perm
            rv, sv = rvsv[key]
            co["st_" + key] = _core_edge_stats(rv, sv, members,
                                               slot_of_global)
        cores.append(co)

    # per-tile uniform width = max degree over the tile, all cores;
    # then quantized so each supergroup gets one uniform k
    k_sched = {}
    for key in ("s", "d"):
        ks = []
        for t in range(nt):
            g0, g1 = t * P, (t + 1) * P
            kmax = 2
            for co in cores:
                _, _, _, deg = co["st_" + key]
                kmax = max(kmax, int(deg[g0:g1].max()))
            ks.append(kmax)
        k_sched[key] = _quantize(ks)

    for co in cores:
        for key in ("s", "d"):
            slot, s, start, deg = co.pop("st_" + key)
            kmax = max(k_sched[key])
            ell = np.full((npp, kmax), -1, np.int64)
            pos = np.arange(s.shape[0], dtype=np.int64) - start[slot]
            ell[slot, pos] = s
            first = ell[:, 0].copy()
            first[first < 0] = 0
            m = ell < 0
            ell[m] = np.broadcast_to(first[:, None], ell.shape)[m]
            idx_chunks, msk_chunks = [], []
            for t in range(nt):
                k = k_sched[key][t]
                blk = ell[t * P:(t + 1) * P, :k]          # [P, k]
                half = CFG.pair_rows
                idx_chunks.append(np.ascontiguousarray(
                    (blk % half).T).reshape(-1))          # [k, P] flat
                msk_chunks.append((blk >= half).T)        # [k, P]
            co["idx_" + key] = _wrap16(np.concatenate(idx_chunks))
            # expanded select mask: [P, Stot, 64] -> [P, Stot*64] f16
            mk = np.concatenate(msk_chunks, axis=0)       # [Stot, P]
            mke = np.repeat(mk.T.astype(np.float16)[:, :, None], 64,
                            axis=2)
            co["msk_" + key] = np.ascontiguousarray(
                mke.reshape(P, -1))                       # [128, Stot*64]
            degc = np.maximum(deg, 1).astype(np.float32)
            co["deg_" + key] = np.ascontiguousarray(
                degc.reshape(nt, P).T).astype(np.float32)     # [128, nt]
            sch = np.asarray(k_sched[key], np.int64)
            padc = (sch[:, None] - deg.reshape(nt, P)).T.astype(np.float32)
            co["pad_" + key] = np.ascontiguousarray(padc)     # [128, nt]
            xp = np.zeros((npp, D), np.float32)
            valid = co["glob_perm_" + key] >= 0
            xp[valid] = x[co["glob_perm_" + key][valid]]
            co["xperm_" + key] = xp

    xT = np.zeros((D, CFG.ntot), np.float16)
    xT[:, :nn] = x.T.astype(np.float16)
    return cores, k_sched, xT


def _blockdiag(w):  # w: [T, FI, FO] -> [T*FI, T*FO]
    t, fi, fo = w.shape
    out = np.zeros((t * fi, t * fo), np.float32)
    for i in range(t):
        out[i * fi:(i + 1) * fi, i * fo:(i + 1) * fo] = w[i]
    return out


def _weights_prep(inp):
    """Pure re-layout of the input weights (no arithmetic)."""
    w = {}
    for dk in ("s2d", "d2s"):
        preW = np.asarray(inp["pre_W_" + dk], np.float32)   # [T, 2FI, FI]
        preb = np.asarray(inp["pre_b_" + dk], np.float32).reshape(-1)  # [64]
        WA = _blockdiag(preW[:, :FI, :])                    # [64, 64]
        WB = _blockdiag(preW[:, FI:, :])                    # [64, 64]
        dup = np.zeros((65, 128), np.float32)
        dup[:64, :64] = WA
        dup[:64, 64:] = WA
        dup[64, :64] = preb
        dup[64, 64:] = preb
        half = np.zeros((65, 128), np.float32)
        half[:64, :64] = WA
        half[64, :64] = preb
        w["WAdup_" + dk] = dup
        w["WAhalf_" + dk] = half
        w["WB_" + dk] = WB
        postW = np.asarray(inp["post_W_" + dk], np.float32)  # [T, 208, 16]
        P0 = _blockdiag(postW[:, 0:FI, :])                   # [64, 64]
        Ps = []
        for blk in range(3):                                 # 1, amp, 1/amp
            Pg = np.zeros((256, 64), np.float32)
            for a in range(4):                               # mean/mn/mx/std
                for t in range(T):
                    rows = FI + blk * 4 * FI + a * FI
                    Pg[a * 64 + t * FI:a * 64 + (t + 1) * FI,
                       t * FI:(t + 1) * FI] = postW[t, rows:rows + FI, :]
            Ps.append(Pg)
        w["P0T_" + dk] = np.ascontiguousarray(P0.T)          # [64, 64]
        for i, Pg in enumerate(Ps):
            w[f"P{i+1}T_{dk}"] = np.ascontiguousarray(Pg.T)  # [64, 256]
        w["linW_" + dk] = np.asarray(inp["lin_W_" + dk], np.float32)
        w["linb_" + dk] = np.asarray(
            inp["lin_b_" + dk], np.float32).reshape(1, 64)
        w["postb_col_" + dk] = np.asarray(
            inp["post_b_" + dk], np.float32).reshape(64, 1)
    wbp = np.zeros((64, 128), np.float32)
    wbp[:, :64] = w["WB_s2d"]
    wbp[:, 64:] = w["WB_d2s"]
    w["WBpair"] = wbp
    w["selfW"] = np.asarray(inp["lin_self_W"], np.float32)
    w["selfb"] = np.asarray(inp["lin_self_b"], np.float32).reshape(1, 64)
    w["alpha"] = np.asarray(inp["alpha"], np.float32).reshape(1, 1)
    return w


# --------------------------------------------------------------------------
# Device kernel
# --------------------------------------------------------------------------

WEIGHT_SPECS = [
    ("WBpair", (64, 128)),
    ("WAdup_s2d", (65, 128)), ("WAdup_d2s", (65, 128)),
    ("WAhalf_s2d", (65, 128)), ("WAhalf_d2s", (65, 128)),
    ("P0T_s2d", (64, 64)), ("P0T_d2s", (64, 64)),
    ("P1T_s2d", (64, 256)), ("P1T_d2s", (64, 256)),
    ("P2T_s2d", (64, 256)), ("P2T_d2s", (64, 256)),
    ("P3T_s2d", (64, 256)), ("P3T_d2s", (64, 256)),
    ("linW_s2d", (64, 64)), ("linW_d2s", (64, 64)),
    ("linb_s2d", (1, 64)), ("linb_d2s", (1, 64)),
    ("postb_col_s2d", (64, 1)), ("postb_col_d2s", (64, 1)),
    ("selfW", (64, 64)), ("selfb", (1, 64)),
    ("alpha", (1, 1)),
]
COL_NAMES = ["deg", "pad"]


def _emit_tree(nc, pool, vsl, gw, k, out_f32, op, tag):
    """Run-wide min/max tree over vsl(a, b) -> AP [128, gw, b-a, 64]
    (f16). Overlap-pairing (idempotent ops) avoids odd-element carries."""
    if k == 1:
        nc.vector.tensor_copy(out=out_f32, in_=vsl(0, 1)[:, :, 0, :])
        return
    if k == 2:
        nc.vector.tensor_tensor(out=out_f32, in0=vsl(0, 1)[:, :, 0, :],
                                in1=vsl(1, 2)[:, :, 0, :], op=op)
        return
    h = (k + 1) // 2
    tmp = pool.tile([P, gw, max(2, (k + 1) // 2), 64], F16, tag=tag,
                    name=tag, bufs=1)
    nc.vector.tensor_tensor(out=tmp[:, :, :h, :], in0=vsl(0, h),
                            in1=vsl(k - h, k), op=op)
    m = h
    while m > 2:
        h = (m + 1) // 2
        nc.vector.tensor_tensor(out=tmp[:, :, :h, :], in0=tmp[:, :, :h, :],
                                in1=tmp[:, :, m - h:m, :], op=op)
        m = h
    nc.vector.tensor_tensor(out=out_f32, in0=tmp[:, :, 0, :],
                            in1=tmp[:, :, 1, :], op=op)


def _emit_sum_tree(nc, pool, first_in, k, out_f32, tag):
    """Run-wide exact sum tree over first_in(a, b) -> [P, gw, b-a, 64]
    (f16 source)."""
    gw = out_f32.shape[1]
    if k == 2:
        nc.vector.tensor_tensor(out=out_f32, in0=first_in(0, 1)[:, :, 0, :],
                                in1=first_in(1, 2)[:, :, 0, :], op=ALU.add)
        return
    if k == 3:
        nc.vector.tensor_tensor(out=out_f32, in0=first_in(0, 1)[:, :, 0, :],
                                in1=first_in(1, 2)[:, :, 0, :], op=ALU.add)
        nc.vector.tensor_tensor(out=out_f32, in0=out_f32,
                                in1=first_in(2, 3)[:, :, 0, :], op=ALU.add)
        return
    m = k // 2
    tmpb = pool.tile([P, gw, m, 64], F32, tag=tag, name=tag, bufs=1)
    nc.vector.tensor_tensor(out=tmpb[:, :, :m, :], in0=first_in(0, m),
                            in1=first_in(m, 2 * m), op=ALU.add)
    while m > 2:
        h, odd = m // 2, m % 2
        nc.vector.tensor_tensor(out=tmpb[:, :, :h, :], in0=tmpb[:, :, :h, :],
                                in1=tmpb[:, :, h + odd:m, :], op=ALU.add)
        m = h + odd
    nc.vector.tensor_tensor(out=out_f32, in0=tmpb[:, :, 0, :],
                            in1=tmpb[:, :, 1, :], op=ALU.add)
    if k % 2:
        nc.vector.tensor_tensor(out=out_f32, in0=out_f32,
                                in1=first_in(k - 1, k)[:, :, 0, :],
                                op=ALU.add)


def build_kernel(k_sched):
    nt, ntot, npc_pad = CFG.nt, CFG.ntot, CFG.npc_pad
    nc = bacc.Bacc("TRN2", target_bir_lowering=False, debug=False,
                   num_devices=CFG.n_cores)

    din = {}
    din["xT_f"] = nc.dram_tensor("xT_f", [64, ntot], F16,
                                 kind="ExternalInput").ap()
    y_dram = {}
    for key in ("s", "d"):
        din["xperm_" + key] = nc.dram_tensor(
            "xperm_" + key, [npc_pad, 64], F32, kind="ExternalInput").ap()
        stot = sum(k_sched[key])
        din["idx_" + key] = nc.dram_tensor(
            "idx_" + key, [P, 8 * stot], I16, kind="ExternalInput").ap()
        din["msk_" + key] = nc.dram_tensor(
            "msk_" + key, [P, stot * 64], F16, kind="ExternalInput").ap()
        for nm in COL_NAMES:
            din[f"{nm}_{key}"] = nc.dram_tensor(
                f"{nm}_{key}", [P, nt], F32, kind="ExternalInput").ap()
        y_dram[key] = nc.dram_tensor("y_" + key, [npc_pad, 64], F32,
                                     kind="ExternalOutput").ap()
    for nm, shp in WEIGHT_SPECS:
        din[nm] = nc.dram_tensor(nm, list(shp), F32,
                                 kind="ExternalInput").ap()
    btab = {
        "s": nc.dram_tensor("btab_s", [CFG.pair_rows, 128], F16,
                            kind="Internal").ap(),
        "d": nc.dram_tensor("btab_d", [CFG.pair_rows, 128], F16,
                            kind="Internal").ap(),
    }

    with tile.TileContext(nc) as tc:
        _emit(tc, nc, din, y_dram, btab, k_sched)

    nc.compile()
    return nc


def _emit(tc, nc, din, y_dram, btab, k_sched):
    nt, ntot = CFG.nt, CFG.ntot
    ctx = ExitStack()
    consts = ctx.enter_context(tc.tile_pool(name="consts", bufs=1))
    small = ctx.enter_context(tc.tile_pool(name="small", bufs=3))
    work = ctx.enter_context(tc.tile_pool(name="work", bufs=2))

    # ---- constants ------------------------------------------------------
    ident = consts.tile([P, P], F32)
    make_identity(nc, ident[:])
    eps_b = consts.tile([P, 1], F32)
    nc.vector.memset(eps_b[:], 1e-5)

    w_sb = {}
    for nm, shp in WEIGHT_SPECS:
        t = consts.tile([shp[0], shp[1]], F32, tag="w_" + nm)
        nc.sync.dma_start(out=t[:], in_=din[nm][:, :])
        w_sb[nm] = t

    # preload ALL gather indices (both directions) once
    idx_sb = {}
    for key in ("s", "d"):
        stot = sum(k_sched[key])
        it = consts.tile([P, 8 * stot], I16, tag="idx" + key,
                         name="idx" + key)
        nc.sync.dma_start(out=it[:], in_=din["idx_" + key][:, :])
        idx_sb[key] = it

    cols = {}
    amps, invamps, invdegs = {}, {}, {}
    for key in ("s", "d"):
        for nm in COL_NAMES:
            cname = f"{nm}_{key}"
            ct = consts.tile([P, nt], F32, tag=cname, name=cname)
            nc.sync.dma_start(out=ct[:], in_=din[cname][:, :])
            cols[cname] = ct
        amps[key] = consts.tile([P, nt], F32, tag="amp" + key,
                                name="amp" + key)
        nc.scalar.activation(out=amps[key][:], in_=cols["deg_" + key][:],
                             func=ACTF.Ln, bias=1.0, scale=1.0)
        invamps[key] = consts.tile([P, nt], F32, tag="iamp" + key,
                                   name="iamp" + key)
        nc.vector.reciprocal(out=invamps[key][:], in_=amps[key][:])
        invdegs[key] = consts.tile([P, nt], F32, tag="ideg" + key,
                                   name="ideg" + key)
        nc.vector.reciprocal(out=invdegs[key][:], in_=cols["deg_" + key][:])

    # ---- alpha, scaled linW, G matrices, bias ---------------------------
    alpha_b = consts.tile([64, 1], F32)
    nc.gpsimd.dma_start(
        out=alpha_b[:],
        in_=bass.AP(tensor=din["alpha"].tensor, offset=0,
                    ap=[[0, 64], [1, 1]]))
    a_d2s = alpha_b
    a_s2d = consts.tile([64, 1], F32)
    nc.vector.memset(a_s2d[:], 1.0)
    nc.vector.tensor_sub(out=a_s2d[:], in0=a_s2d[:], in1=alpha_b[:])

    alph = {"s": a_s2d, "d": a_d2s}
    dk_of = {"s": "s2d", "d": "d2s"}
    linWs = {}
    for key in ("s", "d"):
        lw = consts.tile([64, 64], F32, tag="linWs" + key, name="linWs" + key)
        nc.vector.tensor_scalar_mul(
            out=lw[:], in0=w_sb["linW_" + dk_of[key]][:], scalar1=alph[key][:])
        linWs[key] = lw

    G = {}
    G0 = {}
    selfW_ext = consts.tile([65, 64], F32)
    nc.sync.dma_start(out=selfW_ext[:64, :], in_=din["selfW"][:, :])

    wbpair16 = consts.tile([64, 128], F16, tag="wbpair16")
    nc.vector.tensor_copy(out=wbpair16[:], in_=w_sb["WBpair"][:])

    scale_of = {1: 1.0, 2: 1.0 / AVG_LOG, 3: AVG_LOG}
    with tc.tile_pool(name="setup_ps", bufs=4, space="PSUM") as setup_ps:
        # ---- phase 0: pair-packed B tables ------------------------------
        # btab_<dir>[r, :] = [B[r] | B[r + 25088]]; a 1024-node chunk
        # writes node-major with contiguous 128 B runs. Loads on sync,
        # matmuls on PE, f16 casts + table writes on ACT.
        CH = 1024
        n_chunks = ntot // CH
        half = CFG.pair_rows
        ph0_cm = tc.tile_pool(name="ph0", bufs=1)
        ph0 = ph0_cm.__enter__()
        # build btab_s for ALL nodes first (s-gathers can then start),
        # then btab_d underneath the early s-gathers.
        for key, p0 in (("s", 0), ("d", 64)):
            for ci in range(n_chunks):
                xch = ph0.tile([64, CH], F16, tag="xch", bufs=3)
                nc.scalar.dma_start(out=xch[:],
                                    in_=din["xT_f"][:, ci * CH:(ci + 1) * CH])
                ps_big = setup_ps.tile([P, CH // 2], F32, tag="bps",
                                       name="bps", bufs=2)
                for j in range(CH // P):
                    nc.tensor.matmul(out=ps_big[:, j * 64:(j + 1) * 64],
                                     lhsT=xch[:, j * P:(j + 1) * P],
                                     rhs=wbpair16[:, p0:p0 + 64],
                                     start=True, stop=True)
                nj = CH // P
                bsb = ph0.tile([P, nj, 64], F16, tag="bsb", bufs=3)
                nc.scalar.copy(
                    out=bsb[:, :, :],
                    in_=ps_big[:].rearrange("p (j c) -> p j c", j=nj))
                pieces = []
                n0 = ci * CH
                jmid = (half - n0) // P
                if jmid <= 0:
                    pieces.append((0, nj, n0 - half, 64))
                elif jmid >= nj:
                    pieces.append((0, nj, n0, 0))
                else:
                    pieces.append((0, jmid, n0, 0))
                    pieces.append((jmid, nj, n0 + jmid * P - half, 64))
                for j0, j1, row0, c0 in pieces:
                    out_ap = bass.AP(tensor=btab[key].tensor,
                                     offset=row0 * 128 + c0,
                                     ap=[[128, 128], [128 * 128, j1 - j0],
                                         [1, 64]])
                    nc.sync.dma_start(out=out_ap,
                                       in_=bsb[:, j0:j1, :])
        ph0_cm.__exit__(None, None, None)
        for key in ("s", "d"):
            dk = dk_of[key]
            for i in (1, 2, 3):
                for c in (0, 1):
                    ps = setup_ps.tile([P, 64], F32, tag="gps", name="gps",
                                       bufs=1)
                    nc.tensor.matmul(
                        out=ps[:],
                        lhsT=w_sb[f"P{i}T_{dk}"][:, c * P:(c + 1) * P],
                        rhs=linWs[key][:], start=True, stop=True)
                    g = consts.tile([P, 64], F32, tag=f"G{i}{key}{c}",
                                    name=f"G{i}{key}{c}")
                    nc.scalar.activation(out=g[:], in_=ps[:], func=ACTF.Copy,
                                         scale=scale_of[i])
                    G[f"{i}{key}{c}"] = g
            ps = setup_ps.tile([64, 64], F32, tag="g0ps", name="g0ps",
                             bufs=1)
            nc.tensor.matmul(out=ps[:], lhsT=w_sb[f"P0T_{dk}"][:],
                             rhs=linWs[key][:], start=True, stop=True)
            g0 = consts.tile([P, 64], F32, tag="G0" + key, name="G0" + key)
            nc.vector.tensor_copy(out=g0[:64, :], in_=ps[:])
            G0[key] = g0

        bias_ps = setup_ps.tile([1, 64], F32, tag="biasps",
                                name="biasps", bufs=1)
        nc.tensor.matmul(out=bias_ps[:], lhsT=w_sb["postb_col_s2d"][:],
                         rhs=linWs["s"][:], start=True, stop=False)
        nc.tensor.matmul(out=bias_ps[:], lhsT=w_sb["postb_col_d2s"][:],
                         rhs=linWs["d"][:], start=False, stop=True)
        tb = small.tile([1, 64], F32, tag="tb")
        nc.vector.tensor_scalar_mul(out=tb[:], in0=w_sb["linb_s2d"][:],
                                    scalar1=a_s2d[:1, :])
        nc.vector.tensor_add(out=tb[:], in0=tb[:], in1=bias_ps[:])
        tb2 = small.tile([1, 64], F32, tag="tb2")
        nc.vector.tensor_scalar_mul(out=tb2[:], in0=w_sb["linb_d2s"][:],
                                    scalar1=a_d2s[:1, :])
        nc.vector.tensor_add(out=tb[:], in0=tb[:], in1=tb2[:])
        nc.vector.tensor_add(out=tb[:], in0=tb[:], in1=w_sb["selfb"][:])
        nc.sync.dma_start(out=selfW_ext[64:65, :], in_=tb[:])

    # ---- main loop ------------------------------------------------------
    psum = ctx.enter_context(tc.tile_pool(name="psum", bufs=1, space="PSUM"))


    def bcast(col_ap, gw):
        # [128, gw] column slice -> [128, gw, 64] free-broadcast AP
        return col_ap.unsqueeze(2).to_broadcast([P, gw, 64])

    pend = []

    def _emit_pe(item):
        key, tiles, ag, k = item
        for ti, t in enumerate(tiles):
            xp = small.tile([P, 64], F32, tag="xp")
            nc.scalar.dma_start(
                out=xp[:],
                in_=din["xperm_" + key][t * P:(t + 1) * P, :])
            xpT_ps = psum.tile([64, P], F32, tag="tp",
                               name="xpT_ps", bufs=2)
            nc.tensor.transpose(out=xpT_ps[:], in_=xp[:],
                                identity=ident[:])
            xpT32 = small.tile([65, P], F32, tag="xpT32")
            nc.scalar.copy(out=xpT32[:64, :], in_=xpT_ps[:])
            nc.vector.memset(xpT32[64:65, :], 1.0)

            u1 = psum.tile([64, P], F32, tag="u1", name="u1" + key,
                           bufs=2)
            aggT = work.tile([P, 2, P], F32, tag="aggT",
                             name="aggT" + key)
            for c, wkind in ((0, "dup"), (1, "half")):
                tp = psum.tile([P, P], F32, tag="tp", name="tp",
                               bufs=2)
                nc.tensor.matmul(out=tp[:],
                                 lhsT=ag[:, ti, 2 * c:2 * c + 2, :],
                                 rhs=ident[:], is_transpose=True,
                                 start=True, stop=False,
                                 skip_group_check=True)
                nc.tensor.matmul(
                    out=tp[:],
                    lhsT=w_sb[f"WA{wkind}_{dk_of[key]}"][:],
                    rhs=xpT32[:], start=False, stop=True,
                    skip_group_check=True)
                nc.scalar.copy(out=aggT[:, c, :], in_=tp[:])

            nc.tensor.matmul(out=u1[:], lhsT=G[f"1{key}0"][:],
                             rhs=aggT[:, 0, :], start=True,
                             stop=False, skip_group_check=True)
            nc.tensor.matmul(out=u1[:], lhsT=G[f"1{key}1"][:],
                             rhs=aggT[:, 1, :], start=False,
                             stop=False, skip_group_check=True)
            nc.tensor.matmul(out=u1[:], lhsT=G0[key][:64, :],
                             rhs=xpT32[:64, :], start=False,
                             stop=(key == "d"),
                             skip_group_check=True)
            if key == "s":
                nc.tensor.matmul(out=u1[:], lhsT=selfW_ext[:],
                                 rhs=xpT32[:], start=False,
                                 stop=True, skip_group_check=True)
            u23 = psum.tile([P, P], F32, tag="u23",
                            name="u23" + key, bufs=2)
            nc.tensor.matmul(out=u23[:64, :], lhsT=G[f"2{key}0"][:],
                             rhs=aggT[:, 0, :], start=True,
                             stop=False, skip_group_check=True)
            nc.tensor.matmul(out=u23[:64, :], lhsT=G[f"2{key}1"][:],
                             rhs=aggT[:, 1, :], start=False,
                             stop=True, skip_group_check=True)
            nc.tensor.matmul(out=u23[64:, :], lhsT=G[f"3{key}0"][:],
                             rhs=aggT[:, 0, :], start=True,
                             stop=False, skip_group_check=True)
            nc.tensor.matmul(out=u23[64:, :], lhsT=G[f"3{key}1"][:],
                             rhs=aggT[:, 1, :], start=False,
                             stop=True, skip_group_check=True)

            u1sb = small.tile([64, P], F32, tag="u1sb")
            nc.scalar.copy(out=u1sb[:], in_=u1[:])
            u1T = psum.tile([P, 64], F32, tag="utr", name="u1T",
                            bufs=2)
            nc.tensor.transpose(out=u1T[:], in_=u1sb[:],
                                identity=ident[:64, :64])
            upk = small.tile([P, P], F32, tag="upk",
                             name="upk" + key)
            nc.scalar.copy(out=upk[:], in_=u23[:])
            uT = psum.tile([P, P], F32, tag="utr", name="uT" + key,
                           bufs=2)
            nc.tensor.transpose(out=uT[:], in_=upk[:],
                                identity=ident[:])
            y_sb = small.tile([P, 64], F32, tag="y_sb")
            sc = small.tile([P, 64], F32, tag="sc", name="sc" + key)
            nc.scalar.activation(out=sc[:], in_=uT[:, 0:64],
                                 func=ACTF.Copy,
                                 scale=amps[key][:, t:t + 1])
            nc.vector.tensor_add(out=y_sb[:], in0=u1T[:], in1=sc[:])
            nc.scalar.activation(out=sc[:], in_=uT[:, 64:128],
                                 func=ACTF.Copy,
                                 scale=invamps[key][:, t:t + 1])
            nc.vector.tensor_add(out=y_sb[:], in0=y_sb[:],
                                 in1=sc[:])
            nc.sync.dma_start(
                out=y_dram[key][t * P:(t + 1) * P, :], in_=y_sb[:])

    for key in ("s", "d"):
        sgs = _supergroups(k_sched[key])
        offs = []
        o = 0
        for sg in sgs:
            offs.append(o)
            o += sum(k_sched[key][t] for t in sg)
        # descending-k pipeline, but finish with the few-tile head groups
        # so the post-gather tail (trees + PE phase) is short
        order = list(zip(sgs, offs))
        order = order[2:] + order[:2][::-1]
        for sg, off in order:
            S = sum(k_sched[key][t] for t in sg)
            v = work.tile([P, S, 128], F16, tag="vg", name="vg" + key,
                          bufs=3)
            nc.gpsimd.dma_gather(
                out_ap=v[:, :, :], in_ap=btab[key][:, :],
                idxs_ap=idx_sb[key][:, 8 * off:8 * (off + S)],
                num_idxs=P * S, num_idxs_reg=P * S, elem_size=128,
                single_packet=False)

            mskt = work.tile([P, S, 64], F16, tag="msk", name="msk" + key,
                             bufs=3)
            nc.sync.dma_start(
                out=mskt[:],
                in_=din["msk_" + key][:, off * 64:(off + S) * 64]
                .rearrange("p (s f) -> p s f", f=64))

            # half-select: vd = v_lo + m*(v_hi - v_lo), full-rate DVE
            vd = work.tile([P, S, 64], F16, tag="vd", name="vd" + key,
                           bufs=2)
            nc.vector.tensor_sub(out=vd[:, :, :], in0=v[:, :, 64:128],
                                 in1=v[:, :, 0:64])
            nc.vector.tensor_tensor(out=vd[:, :, :], in0=vd[:, :, :],
                                    in1=mskt[:, :, :], op=ALU.mult)
            nc.vector.tensor_tensor(out=vd[:, :, :], in0=vd[:, :, :],
                                    in1=v[:, :, 0:64], op=ALU.add)

            if len(pend) > 0:
                _emit_pe(pend.pop(0))

            gw = len(sg)
            k = k_sched[key][sg[0]]
            g0 = sg[0]
            if True:
                tiles = sg
                vd4 = vd[:, :, :].rearrange(
                    "p (t k) f -> p t k f", t=gw)

                def vsl(a, b, vd4=vd4):
                    return vd4[:, :, a:b, :]

                v2 = work.tile([P, gw, k, 64], F16, tag="v2",
                               name="v2" + key, bufs=1)
                nc.scalar.activation(out=v2[:, :, :, :], in_=vd4,
                                     func=ACTF.Square)
                v24 = v2[:, :, :, :]

                def vsl2(a, b, v24=v24):
                    return v24[:, :, a:b, :]

                s_ = work.tile([P, gw, 64], F32, tag="s_", name="s_" + key)
                _emit_sum_tree(nc, work, vsl, k, s_[:, :, :], "st")
                q_ = work.tile([P, gw, 64], F32, tag="q_", name="q_" + key)
                _emit_sum_tree(nc, work, vsl2, k, q_[:, :, :], "st")

                # pad compensation (pads replicate slot 0)
                tmp = work.tile([P, gw, 64], F32, tag="tmp",
                                name="tmp" + key)
                gsl = slice(g0, g0 + gw)
                padb = bcast(cols[f"pad_{key}"][:, gsl], gw)
                nc.vector.tensor_tensor(out=tmp[:, :, :],
                                        in0=vd4[:, :, 0, :],
                                        in1=padb, op=ALU.mult)
                nc.vector.tensor_sub(out=s_[:, :, :], in0=s_[:, :, :],
                                     in1=tmp[:, :, :])
                nc.vector.tensor_tensor(out=tmp[:, :, :],
                                        in0=v2[:, :, 0, :],
                                        in1=padb, op=ALU.mult)
                nc.vector.tensor_sub(out=q_[:, :, :], in0=q_[:, :, :],
                                     in1=tmp[:, :, :])

                ag = work.tile([P, gw, 4, 64], F32, tag="aggG" + key,
                               name="aggG" + key)
                idg = bcast(invdegs[key][:, gsl], gw)
                nc.vector.tensor_tensor(out=ag[:, :, 0, :],
                                        in0=s_[:, :, :], in1=idg,
                                        op=ALU.mult)
                nc.vector.tensor_tensor(out=q_[:, :, :], in0=q_[:, :, :],
                                        in1=idg, op=ALU.mult)
                nc.vector.tensor_tensor(out=tmp[:, :, :],
                                        in0=ag[:, :, 0, :],
                                        in1=ag[:, :, 0, :], op=ALU.mult)
                nc.vector.tensor_sub(out=q_[:, :, :], in0=q_[:, :, :],
                                     in1=tmp[:, :, :])
                nc.vector.tensor_scalar_max(out=q_[:, :, :],
                                            in0=q_[:, :, :], scalar1=0.0)
                nc.scalar.activation(out=ag[:, :, 3, :], in_=q_[:, :, :],
                                     func=ACTF.Sqrt, bias=eps_b[:],
                                     scale=1.0)

                _emit_tree(nc, work, vsl, gw, k, ag[:, :, 1, :], ALU.min,
                           "tr")
                _emit_tree(nc, work, vsl, gw, k, ag[:, :, 2, :], ALU.max,
                           "tr")

                # ---- defer PE phase by one supergroup ----
                pend.append((key, tiles, ag, k))

    while pend:
        _emit_pe(pend.pop(0))

    ctx.close()


# --------------------------------------------------------------------------
# Entry point
# --------------------------------------------------------------------------

_CACHE = {}


def make_in_maps(inputs):
    x = np.asarray(inputs["x"], np.float32)
    ei = np.asarray(inputs["edge_index"])
    cores, k_sched, xT = _host_prep(x, ei)
    w = _weights_prep(inputs)
    in_maps = []
    for co in cores:
        m = {"xT_f": xT}
        for key in ("s", "d"):
            m["xperm_" + key] = co["xperm_" + key]
            m["idx_" + key] = co["idx_" + key]
            m["msk_" + key] = co["msk_" + key]
            for nm in COL_NAMES:
                m[f"{nm}_{key}"] = co[f"{nm}_{key}"]
        for nm, shp in WEIGHT_SPECS:
            m[nm] = np.ascontiguousarray(w[nm].reshape(shp))
        in_maps.append(m)
    return cores, k_sched, in_maps


def kernel(**inputs):
    configure(int(np.asarray(inputs["x"]).shape[0]))
    cores, k_sched, in_maps = make_in_maps(inputs)

    key = (CFG.n_nodes, tuple(k_sched["s"]), tuple(k_sched["d"]))
    if key not in _CACHE:
        _CACHE[key] = build_kernel(k_sched)
    nc = _CACHE[key]

    res = bass_utils.run_bass_kernel_spmd(
        nc, in_maps, core_ids=list(range(CFG.n_cores)))

    y_full = np.zeros((CFG.n_nodes, D), np.float32)
    for key in ("s", "d"):
        for c, co in enumerate(cores):
            yc = res.results[c]["y_" + key]
            perm = co["glob_perm_" + key]
            valid = perm >= 0
            y_full[perm[valid]] += yc[valid]
    return y_full
